# revision 45
# baseline (speedup 1.0000x reference)
"""Trainium2 Bass kernel for nn_Attention_6794638262338.

Single-layer attention block with BitNet-style ternary-quantized projections:
    x -> LN1 -> qkv proj (ternary W) -> MHA softmax -> LN2 -> out proj (ternary W)

Strategy: pure data parallelism. batch=8, n_cores=8 -> one batch element per
core, no collectives. Each core runs an identical Bass/Tile program.

Math folds (host side):
  - ternary_quant(W) = T * s with T in {-1,0,1}: pass T in bf16 (exact), fold
    s_qkv^2 * DIM_HEAD^-0.5 into the exp() activation scale, fold s_qkv/s_out
    into the LN2 rsqrt epsilon/scale.
  - softmax denominator: out = (sum_m exp(s)*v) / colsum. colsum obtained free
    by appending a ones-column to v in the attn@v matmul (M=65); division done
    via DVE reciprocal + GpSimd partition_broadcast + DVE multiply.
  - LN2: mean/var via ones-matmul column sums of a^T, tiny PE transposes to get
    per-row stats, y = (z - mu*W1) * rsqrt-ish using host-precomputed
    W1 = rowsum of effective output weight.
"""

import numpy as np
from contextlib import ExitStack

import concourse.bass as bass
import concourse.mybir as mybir
import concourse.tile as tile
from concourse import bacc
from concourse.bass import ts, ds
from concourse.bass_utils import run_bass_kernel_spmd
from concourse.masks import make_identity

F32 = mybir.dt.float32
BF16 = mybir.dt.bfloat16
FP8 = mybir.dt.float8e4
DR = mybir.MatmulPerfMode.DoubleRow
AF = mybir.ActivationFunctionType
ALU = mybir.AluOpType

B, N, D = 8, 1024, 512
H, DH = 8, 64
INNER = H * DH  # 512
NT = N // 128   # 8 n-tiles
DC = D // 128   # 4 d-chunks
EPS_LN = 1e-5
EPS_Q = 1e-6

TRACE = False          # set by test.py to capture an NTFF profile
LAST_RESULTS = None    # BassKernelResults of the most recent run

_CACHE = {}


def _ternary(w):
    """Replicate reference ternary_quant in fp32; return (unit ternary, scale)."""
    w = np.asarray(w, np.float32)
    s = np.float32(np.mean(np.abs(w), dtype=np.float32))
    t = np.round(np.clip(w / (s + np.float32(EPS_Q)), -1.0, 1.0)).astype(np.float32)
    return t, float(s)


def _emit(ctx: ExitStack, tc: "tile.TileContext", io: dict, c: dict, sfx: str = ""):
    nc = tc.nc
    dbg = c.get("debug", False)
    loop_reps = c.get("loop_reps", 0)

    def dump(name, ap):
        if dbg:
            d = nc.dram_tensor(f"dbg_{name}{sfx}", list(ap.shape), ap.dtype, kind="ExternalOutput").ap()
            nc.sync.dma_start(out=d, in_=ap)
    x, tqT, toT, w1u, y = io["x"], io["tqT"], io["toT"], io["w1u"], io["y"]

    need_g1 = c["need_g1"]
    need_b1 = c["need_b1"]
    need_bt = c["need_bt"]

    # ---------------- pools ----------------
    const_p = ctx.enter_context(tc.tile_pool(name="const" + sfx, bufs=1))
    xp = ctx.enter_context(tc.tile_pool(name="xp" + sfx, bufs=3))
    lnp = ctx.enter_context(tc.tile_pool(name="lnp" + sfx, bufs=4))
    xlnp = ctx.enter_context(tc.tile_pool(name="xlnp" + sfx, bufs=3))
    big = ctx.enter_context(tc.tile_pool(name="big" + sfx, bufs=1))
    attp = ctx.enter_context(tc.tile_pool(name="attp" + sfx, bufs=2))
    smp = ctx.enter_context(tc.tile_pool(name="smp" + sfx, bufs=3))
    outp = ctx.enter_context(tc.tile_pool(name="outp" + sfx, bufs=2))
    # PSUM budget: 8 banks = ps_s ([128,1024] x2 = 4) + ps_o ([65,512] x2 = 2)
    #              + ps_m ([128,512] x2 = 2)
    ps_s = ctx.enter_context(tc.tile_pool(name="ps_s" + sfx, bufs=2, space="PSUM"))
    ps_o = ctx.enter_context(tc.tile_pool(name="ps_o" + sfx, bufs=2, space="PSUM"))
    ps_m = ctx.enter_context(tc.tile_pool(name="ps_m" + sfx, bufs=2, space="PSUM"))

    # ---------------- constants ----------------
    ident = const_p.tile([128, 128], BF16)
    make_identity(nc, ident)
    # stats column: 1/INNER folded in, so the s1/s2 ones-matmuls produce
    # mu and E[a^2] directly (drops two ACT hops per LN2 group)
    ones128 = const_p.tile([128, 1], BF16)
    nc.vector.memset(ones128, 1.0 / INNER)
    eps1 = const_p.tile([128, 1], F32)
    nc.vector.memset(eps1, float(EPS_LN))
    eps2 = const_p.tile([128, 1], F32)
    nc.vector.memset(eps2, c["eps_eff"])
    # warm the ln/exp activation table while the first x tile is in flight
    warm = const_p.tile([128, 1], F32)
    nc.scalar.activation(warm, eps1, AF.Ln, bias=eps1)
    nc.scalar.activation(warm, warm, AF.Exp, scale=-0.5)

    # weight loads go on the GpSimd DMA queue so the x tiles own the SP
    # queue from t=0 (they gate the LN1->transpose critical path)
    # qkv unit-ternary weights, transposed: [d, 3*inner] -> sbuf [128, DC, 3*inner]
    tq_sb = const_p.tile([128, DC, 3 * INNER], BF16)
    nc.gpsimd.dma_start(out=tq_sb, in_=tqT.rearrange("(c p) o -> p c o", p=128))
    # out-proj unit weights (g2 folded), transposed: [o, dout] -> [128, DC, dout]
    toT_sb = const_p.tile([128, DC, INNER], BF16)
    nc.gpsimd.dma_start(out=toT_sb, in_=toT.rearrange("(c p) o -> p c o", p=128))
    # W1 rowsums broadcast across partitions
    w1b = const_p.tile([128, INNER], F32)
    nc.gpsimd.dma_start(
        out=w1b,
        in_=bass.AP(tensor=w1u.tensor, offset=w1u.offset, ap=[[0, 128]] + list(w1u.ap)),
    )
    if need_g1:
        g1_ap = io["g1v"]
        g1b = const_p.tile([128, D], F32)
        nc.gpsimd.dma_start(
            out=g1b,
            in_=bass.AP(tensor=g1_ap.tensor, offset=g1_ap.offset, ap=[[0, 128]] + list(g1_ap.ap)),
        )
    if need_b1:
        b1_ap = io["b1v"]
        b1b = const_p.tile([128, D], F32)
        nc.gpsimd.dma_start(
            out=b1b,
            in_=bass.AP(tensor=b1_ap.tensor, offset=b1_ap.offset, ap=[[0, 128]] + list(b1_ap.ap)),
        )
    if need_bt:
        bt_ap = io["btv"]
        btb = const_p.tile([128, INNER], F32)
        nc.gpsimd.dma_start(
            out=btb,
            in_=bass.AP(tensor=bt_ap.tensor, offset=bt_ap.offset, ap=[[0, 128]] + list(bt_ap.ap)),
        )

    scale_exp = c["scale_exp"]

    def body():
        # ---------------- persistent big tensors ----------------
        # xln^T: [d, n] bf16, split in two n-half tiles [128, DC, 512] so the
        # first qkv matmuls start after only half of Phase A
        xlnTh = [
            big.tile([128, DC, 512], BF16, name=f"xlnTh{i}", tag=f"xlnTh{i}")
            for i in range(2)
        ]
        # q^T, k^T head-major: [o, n] as [128, DC, N] (o = otile*128 + p)
        qT = big.tile([128, DC, N], BF16)
        kT = big.tile([128, DC, N], BF16)
        # v row-major with ones column: [128, mt, h, 65] (m = mt*128 + p)
        v_sb = big.tile([128, NT, H, DH + 1], BF16)
        nc.vector.memset(v_sb[:, :, :, DH : DH + 1], 1.0)
        # pair-stacked divided attention out: partition 0:64 = head 2p,
        # 64:128 = head 2p+1 (DVE cross-partition writes)
        aT2 = big.tile([128, DC, N], BF16)
        # squares of aT2 for the LN2 sum-of-squares (filled by GpSimd)
        sq_sb = big.tile([128, DC, N], BF16)

        def emit_qk(ot, nns=(0, 1)):
            # q, k head-major: psum[o_tile, n] = sum_dc Tq[:,dc,ot].T @ xlnT[:,dc,n]
            # k before q per n-half: the first scores matmul needs (kT nn0,
            # qT nn0) only. Copies run on ACT (Copy activation) to keep DVE
            # free for the fast-exp tiles + divides.
            for nn in nns:
                for sec, dst in ((1, kT), (0, qT)):
                    pq = ps_m.tile([128, 512], F32, name="pq", tag="mm")
                    for dc in range(DC):
                        nc.tensor.matmul(
                            pq,
                            lhsT=tq_sb[:, dc, ds(sec * INNER + ot * 128, 128)],
                            rhs=xlnTh[nn][:, dc, :],
                            start=(dc == 0), stop=(dc == DC - 1),
                        )
                    if c.get("qk_act", True):
                        nc.scalar.activation(out=dst[:, ot, ts(nn, 512)], in_=pq, func=AF.Copy)
                    else:
                        nc.vector.tensor_copy(out=dst[:, ot, ts(nn, 512)], in_=pq)


        # ================ Phase A: load x, LN1, transpose ================
        for nt in range(NT):
            # x arrives bf16 (host-converted): halves the input DMA bytes
            xt = xp.tile([128, D], BF16, name="xt", tag="xt")
            nc.sync.dma_start(out=xt, in_=x[ts(nt, 128), :])
            st6 = lnp.tile([128, 6], F32, name="st6", tag="st6")
            nc.vector.bn_stats(st6, xt)
            mv = lnp.tile([128, 2], F32, name="mv", tag="mv")
            nc.vector.bn_aggr(mv, st6)
            # rstd = exp(-0.5*ln(var+eps)) — keeps ACT on the ln/exp table set
            # (same set the attention exp uses; avoids sqrt-set thrashing)
            sd = lnp.tile([128, 1], F32, name="sd", tag="sd")
            nc.scalar.activation(sd, mv[:, 1:2], AF.Ln, bias=eps1)
            rs = lnp.tile([128, 1], F32, name="rs", tag="rs")
            nc.scalar.activation(rs, sd, AF.Exp, scale=-0.5)
            xl = xlnp.tile([128, D], BF16, name="xl", tag="xl")
            if need_g1 or need_b1:
                xlf = xlnp.tile([128, D], F32, name="xlf", tag="xlf")
                nc.vector.tensor_scalar(
                    out=xlf, in0=xt, scalar1=mv[:, 0:1], scalar2=rs,
                    op0=ALU.subtract, op1=ALU.mult,
                )
                if need_g1:
                    nc.vector.tensor_mul(xlf, xlf, g1b)
                if need_b1:
                    nc.vector.tensor_add(xlf, xlf, b1b)
                nc.vector.tensor_copy(xl, xlf)
            else:
                # (x - mu)*rs on ACT (idle in Phase A; DVE is the pacer):
                # Identity activation with per-partition scale rs and bias
                # -mu*rs
                nrsmu = lnp.tile([128, 1], F32, name="nrsmu", tag="nrsmu")
                nc.vector.tensor_scalar(
                    out=nrsmu, in0=mv[:, 0:1], scalar1=rs, scalar2=-1.0,
                    op0=ALU.mult, op1=ALU.mult,
                )
                nc.scalar.activation(out=xl, in_=xt, func=AF.Identity, scale=rs, bias=nrsmu)
            # transpose via matmul with identity: out = xl_slice.T. All four
            # d-chunks land in one psum tile -> one strided copy into xlnT.
            # Copies for the second half go to GpSimd: DVE is the Phase A
            # pacer and the weight DMAs have drained off the Pool queue by
            # then.
            pt = ps_m.tile([128, DC, 128], F32, name="pt", tag="mm")
            for dc in range(DC):
                nc.tensor.matmul(
                    pt[:, dc, :], lhsT=xl[:, ts(dc, 128)], rhs=ident, start=True, stop=True
                )
            nc.vector.tensor_copy(out=xlnTh[nt // 4][:, :, ts(nt % 4, 128)], in_=pt)
            if nt == 3:
                emit_qk(0, nns=(0,))  # first n-half of q/k as soon as it exists

        # ================ Phase B+C interleaved: qkv otiles feed attention
        # head-pairs as soon as their q/k tile is ready, so ACT starts exp()
        # early and stays the pacer without idle lead-in. ================
        def emit_v(mts):
            # v row-major: psum[m_tile, o] = sum_dc xlnT[:,dc,mt].T @ Tq_v[:,dc,:]
            for mt in mts:
                pv = ps_m.tile([128, 512], F32, name="pv", tag="mm")
                for dc in range(DC):
                    nc.tensor.matmul(
                        pv,
                        lhsT=xlnTh[mt // 4][:, dc, ts(mt % 4, 128)],
                        rhs=tq_sb[:, dc, ds(2 * INNER, INNER)],
                        start=(dc == 0), stop=(dc == DC - 1),
                    )
                # strided copy into per-head layout [128, h, 64]
                nc.vector.tensor_copy(
                    out=v_sb[:, mt, :, 0:DH],
                    in_=pv.rearrange("p (h d) -> p h d", h=H),
                )

        # ~1/3 of the exp tiles run on DVE as a Schraudolph bf16-bit-trick
        # (bits = round(s*scale*log2e*128 + 16256-shift) as int16, bitcast to
        # bf16 ~= exp(s*scale) with ~3% sawtooth err that mostly cancels in
        # softmax). This splits the exp wall (the mid-kernel pacer) between
        # ACT and DVE. Tail-critical (mt=7) tiles stay on ACT.
        fexp_a, fexp_b = c["fexp_a"], c["fexp_b"]
        I16 = mybir.dt.int16

        _fexp_sets = {
            0: {0: (), 1: ()},
            16: {0: (1, 4), 1: (2, 5)},
            24: {0: (1, 3, 5), 1: (2, 4, 6)},
            32: {0: (1, 3, 5, 6), 1: (0, 2, 4, 5)},
            40: {0: (0, 1, 3, 5, 6), 1: (0, 2, 3, 4, 5)},
        }

        def fexp_on_dve(p, sub, mt):
            if not c.get("fexp", True):
                return False
            return mt in _fexp_sets[c.get("fexp_n", 24)][sub]

        def emit_scores_pair(p, interleave=None):
            """Scores+exp for heads 2p (partitions 0:64) and 2p+1 (64:128).
            atn is one tile per (sub, mt) so downstream attn@v matmuls only
            wait on the exps they actually read. `interleave(j)` is called
            after exp(mt=j+3) to slot tail-pair attn@v matmuls between
            score matmuls."""
            atns = {0: {}, 1: {}}
            for mt in range(NT):
                pss = [
                    ps_s.tile([128, N], F32, name="pssa", tag="s"),
                    ps_s.tile([128, N], F32, name="pssb", tag="s"),
                ]
                # sub-major order: each sub's two n-half matmuls are
                # adjacent, so its exp fires one matmul earlier
                for sub in range(2):
                    for nn in range(2):
                        base = sub * 64
                        nc.tensor.matmul(
                            pss[sub][:, ts(nn, 512)],
                            lhsT=kT[ds(base, 64), p, ts(mt, 128)],
                            rhs=qT[ds(base, 64), p, ts(nn, 512)],
                            start=True, stop=True,
                        )
                for sub in range(2):
                    if fexp_on_dve(p, sub, mt):
                        ai = attp.tile([128, N], I16, name=f"atn{sub}_{mt}", tag=f"atn{sub}_{mt}")
                        nc.vector.tensor_scalar(
                            out=ai, in0=pss[sub], scalar1=fexp_a, scalar2=fexp_b,
                            op0=ALU.mult, op1=ALU.add,
                        )
                        a = ai.bitcast(BF16)
                    else:
                        a = attp.tile([128, N], BF16, name=f"atn{sub}_{mt}", tag=f"atn{sub}_{mt}")
                        nc.scalar.activation(out=a, in_=pss[sub], func=AF.Exp, scale=scale_exp)
                    atns[sub][mt] = a
                if interleave is not None and mt >= 3:
                    interleave(mt - 3, atns)
            return atns

        def divide(h, po2):
            # divide chain straight off PSUM: reciprocal of the colsum row
            # (DVE reads psum p64, writes a partition-0 staging row — engines
            # CAN shift partitions, HW-verified) -> GpSimd partition_broadcast
            # (source must be in partitions 0..15: Q7 core 0 does the read)
            # -> multiply straight into aT2 rows 0:64 / 64:128 (cross-
            # partition DVE write kills the old odd-head remap DMA).
            rc0 = smp.tile([1, 2, 512], F32, name="rc0", tag="rc0")
            for nn in range(2):
                nc.vector.reciprocal(rc0[:, nn, :], po2[nn][64:65, :])
            rbt = smp.tile([64, 2, 512], F32, name="rbt", tag="rbt")
            if c.get("pbcast", False):
                nc.gpsimd.partition_broadcast(rbt, rc0, channels=64)
            else:
                bc_dram = nc.dram_tensor(f"cs_scratch{h}" + sfx, [2, 512], F32).ap()
                nc.sync.dma_start(out=bc_dram, in_=rc0)
                nc.sync.dma_start(
                    out=rbt,
                    in_=bass.AP(tensor=bc_dram.tensor, offset=bc_dram.offset,
                                ap=[[0, 64]] + list(bc_dram.ap)),
                )
            for nn in range(2):
                nc.vector.tensor_tensor(
                    out=aT2[ds(64 * (h % 2), 64), h // 2, ts(nn, 512)],
                    in0=po2[nn][0:64, :], in1=rbt[:, nn, :], op=ALU.mult,
                )

        def emit_out(h, atn, pool=None, tag="po"):
            po2 = [
                (pool or ps_o).tile([65, 512], F32, name=f"po{nn}", tag=tag)
                for nn in range(2)
            ]
            for mt in range(NT):
                for nn in range(2):
                    nc.tensor.matmul(
                        po2[nn],
                        lhsT=v_sb[:, mt, h, :],
                        rhs=atn[mt][:, ts(nn, 512)],
                        start=(mt == 0), stop=(mt == NT - 1),
                    )
            divide(h, po2)

        # driver: scores-pair 0 starts as soon as its q/k tile exists (ACT
        # starts exp'ing early); v and the next pair's q/k are emitted behind
        # the current pair's scores so PE fills its exp-wait slack with them;
        # out-matmuls run one pair behind. Squares for the LN2 sum-of-squares
        # run on idle GpSimd as chunks finish (last chunk on DVE: tail-critical).
        emit_qk(0, nns=(1,))  # nn0 was emitted inside Phase A at nt==3
        prev = emit_scores_pair(0)
        emit_v(range(0, 8))
        emit_qk(1)
        for pair in range(1, 3):
            atns = emit_scores_pair(pair)
            emit_qk(pair + 1)
            pp = pair - 1
            emit_out(2 * pp, prev[0])
            emit_out(2 * pp + 1, prev[1])
            nc.gpsimd.tensor_mul(sq_sb[:, pp, :], aT2[:, pp, :], aT2[:, pp, :])
            prev = atns

        # pair 3: head 6's attn@v accumulation is interleaved into the
        # scores loop three exp-steps behind (borrowing the ps_m slots,
        # idle until phase D), so only its last three accumulation steps
        # trail the final exp. Pair 2's out matmuls + divides drain from a
        # work queue a few per step so the PE load stays level and ACT
        # never starves. Head 7 runs after the loop as usual.
        po6 = [ps_m.tile([65, 512], F32, name=f"po6_{nn}", tag="mm") for nn in range(2)]

        po45 = {}
        pending = []

        def _alloc45(h):
            po45[h] = [
                ps_o.tile([65, 512], F32, name=f"po{nn}", tag="po") for nn in range(2)
            ]

        def _mm45(h, atn, mt):
            for nn in range(2):
                nc.tensor.matmul(
                    po45[h][nn],
                    lhsT=v_sb[:, mt, h, :],
                    rhs=atn[mt][:, ts(nn, 512)],
                    start=(mt == 0), stop=(mt == NT - 1),
                )

        for _h, _sub in ((4, 0), (5, 1)):
            pending.append((lambda h=_h: _alloc45(h)))
            for _mt in range(NT):
                pending.append(lambda h=_h, s=_sub, mt=_mt: _mm45(h, prev[s], mt))
            pending.append(lambda h=_h: divide(h, po45[h]))
        pending.append(lambda: nc.gpsimd.tensor_mul(sq_sb[:, 2, :], aT2[:, 2, :], aT2[:, 2, :]))

        def tail_out(j, atns3):
            for nn in range(2):
                nc.tensor.matmul(
                    po6[nn], lhsT=v_sb[:, j, 6, :], rhs=atns3[0][j][:, ts(nn, 512)],
                    start=(j == 0), stop=(j == NT - 1),
                )
            for _ in range(5):
                if pending:
                    pending.pop(0)()

        atns3 = emit_scores_pair(3, interleave=tail_out)
        while pending:
            pending.pop(0)()
        for j in range(5, 8):
            tail_out(j, atns3)
        # head 7 attn@v on ps_s slots (free after the last exps) so it does
        # not wait for head 5's divide to release a ps_o slot
        po7 = [ps_s.tile([65, 512], F32, name=f"po7_{nn}", tag="s") for nn in range(2)]
        for mt in range(NT):
            for nn in range(2):
                nc.tensor.matmul(
                    po7[nn], lhsT=v_sb[:, mt, 7, :], rhs=atns3[1][mt][:, ts(nn, 512)],
                    start=(mt == 0), stop=(mt == NT - 1),
                )
        divide(6, po6)
        divide(7, po7)
        nc.vector.tensor_mul(sq_sb[:, 3, :], aT2[:, 3, :], aT2[:, 3, :])

        dump("qT", qT)
        dump("kT", kT)
        dump("v", v_sb)
        dump("aT2", aT2)

        # ================ Phase D: LN2 stats + output projection ================
        # Four groups of 2 n-tiles. j=0's z goes to a [128, 513] ps_s tile
        # whose 513th column (toT_sb col 512 = 1/INNER) IS the s1 mean —
        # the s1 stats matmuls ride the projection for free. j=1 stays
        # [128, 512] on ps_m (1 bank) + explicit s1 matmuls, preserving the
        # 4-deep pz pipeline. The y ops avoid ACT entirely (Pool + DVE),
        # keeping ACT free for the exp wall.
        for g in range(4):
            # st[:, 0, j] = s1 (sum_o a), st[:, 1, j] = s2 (sum_o a^2)
            st = ps_o.tile([128, 2, 2], F32, name=f"st{g}", tag="po")
            pzs = []
            for j in range(2):
                nt = 2 * g + j
                pz = (
                    ps_s.tile([128, INNER], F32, name="pz", tag="s")
                    if j == 0
                    else ps_m.tile([128, INNER], F32, name="pz", tag="mm")
                )
                pzs.append(pz)
                for ch in range(DC):
                    nc.tensor.matmul(
                        pz, lhsT=aT2[:, ch, ts(nt, 128)], rhs=toT_sb[:, ch, :],
                        start=(ch == 0), stop=(ch == DC - 1),
                    )
                    nc.tensor.matmul(
                        st[:, 0, j : j + 1], lhsT=aT2[:, ch, ts(nt, 128)], rhs=ones128,
                        start=(ch == 0), stop=(ch == DC - 1),
                    )
                for ch in range(DC):
                    nc.tensor.matmul(
                        st[:, 1, j : j + 1], lhsT=sq_sb[:, ch, ts(nt, 128)], rhs=ones128,
                        start=(ch == 0), stop=(ch == DC - 1),
                    )

            # the 1/INNER fold makes mu and E[a^2] direct; var = E[a^2]-mu^2;
            # r2 = s_o / sqrt(var + eps_eff). mu lifted to SBUF right away so
            # psum slots free early.
            muc = lnp.tile([128, 2], F32, name=f"muc{g}", tag="muc", bufs=2)
            nc.vector.tensor_copy(muc, st[:, 0, :])
            musq = lnp.tile([128, 2], F32, name=f"musq{g}", tag="musq", bufs=2)
            nc.vector.tensor_mul(musq, muc, muc)
            var = lnp.tile([128, 2], F32, name=f"var{g}", tag="var", bufs=2)
            nc.vector.tensor_sub(var, st[:, 1, :], musq)
            sd2 = lnp.tile([128, 2], F32, name=f"sd2{g}", tag="sd2", bufs=2)
            nc.scalar.activation(sd2, var, AF.Ln, bias=eps2, scale=c["inv_so2"])
            r2 = lnp.tile([128, 2], F32, name=f"r2{g}", tag="r2", bufs=2)
            nc.scalar.activation(r2, sd2, AF.Exp, scale=-0.5)
            r2n = lnp.tile([128, 2], F32, name=f"r2n{g}", tag="r2n", bufs=2)
            nc.vector.tensor_scalar_mul(r2n, r2, -1.0)
            # nmur2 = -mu*r2 for the rank-1 W1 term
            nmur2 = lnp.tile([128, 2], F32, name=f"nmur2{g}", tag="nmur2", bufs=2)
            nc.vector.tensor_mul(nmur2, muc, r2n)

            # y = (z - mu*W1) * r2 (+ bias_total)
            # j=0: t2 = W1*(-mu*r2) on Pool (idle in tail), then
            #      y = z*r2 + t2 in one DVE op off PSUM.
            # j=1: u = (W1*mu) - z (DVE, z read from PSUM) ; y = u*(-r2)
            for j in range(2):
                nt = 2 * g + j
                # y is written bf16 (halves the output DMA bytes); the last
                # op of each path writes the bf16 tile directly
                yt = outp.tile([128, INNER], BF16, name="yt", tag="yt")
                if j == 0:
                    t2 = outp.tile([128, INNER], F32, name="t2", tag="t2", bufs=2)
                    nc.gpsimd.tensor_scalar(
                        out=t2, in0=w1b, scalar1=nmur2[:, 0:1], scalar2=None,
                        op0=ALU.mult,
                    )
                    if need_bt:
                        ytf = outp.tile([128, INNER], F32, name="ytf", tag="ytf")
                        nc.vector.scalar_tensor_tensor(
                            out=ytf, in0=pzs[0], scalar=r2[:, 0:1],
                            in1=t2, op0=ALU.mult, op1=ALU.add,
                        )
                        nc.vector.tensor_add(yt, ytf, btb)
                    else:
                        nc.vector.scalar_tensor_tensor(
                            out=yt, in0=pzs[0], scalar=r2[:, 0:1],
                            in1=t2, op0=ALU.mult, op1=ALU.add,
                        )
                else:
                    ut = outp.tile([128, INNER], F32, name="ut", tag="ut")
                    nc.vector.scalar_tensor_tensor(
                        out=ut, in0=w1b, scalar=muc[:, 1:2], in1=pzs[1],
                        op0=ALU.mult, op1=ALU.subtract,
                    )
                    if need_bt:
                        nc.vector.tensor_scalar_mul(ut, ut, r2n[:, 1:2])
                        nc.vector.tensor_add(yt, ut, btb)
                    else:
                        nc.vector.tensor_scalar_mul(yt, ut, r2n[:, 1:2])
                nc.sync.dma_start(out=y[ts(nt, 128), :], in_=yt)

    if loop_reps:
        with tc.For_i(0, loop_reps):
            body()
    else:
        body()


def _build(c: dict):
    nc = bacc.Bacc("TRN2", target_bir_lowering=False, debug=False, num_devices=B)
    io = {
        "x": nc.dram_tensor("x", [N, D], BF16, kind="ExternalInput").ap(),
        "tqT": nc.dram_tensor("tqT", [D, 3 * INNER], BF16, kind="ExternalInput").ap(),
        "toT": nc.dram_tensor("toT", [INNER, INNER], BF16, kind="ExternalInput").ap(),
        "w1u": nc.dram_tensor("w1u", [INNER], F32, kind="ExternalInput").ap(),
        "y": nc.dram_tensor("y", [N, D], BF16, kind="ExternalOutput").ap(),
    }
    if c["need_g1"]:
        io["g1v"] = nc.dram_tensor("g1v", [D], F32, kind="ExternalInput").ap()
    if c["need_b1"]:
        io["b1v"] = nc.dram_tensor("b1v", [D], F32, kind="ExternalInput").ap()
    if c["need_bt"]:
        io["btv"] = nc.dram_tensor("btv", [INNER], F32, kind="ExternalInput").ap()
    reps = c.get("body_reps", 1)
    with tile.TileContext(nc) as tc:
        for r in range(reps):
            with ExitStack() as ctx:
                _emit(ctx, tc, io, c, sfx="" if r == 0 else f"_r{r}")

    nc.compile()

    # The act-table-load pass greedily picks the first set containing each
    # function, thrashing between `natural_log` (Ln) and `exp_and_others`
    # (Exp) on every rstd computation (18 reloads @ ~1.3-2.7us each). All
    # activation funcs this kernel uses (Ln, Exp, Copy, Identity) live
    # together in `natural_log_exp_and_others`, so rewrite the first load to
    # that set and drop the rest.
    from concourse.hw_specs import get_activation_tables
    tset = list(get_activation_tables(nc.m.arch).keys())
    nle = tset.index("natural_log_exp_and_others")
    for blk in nc.main_func.blocks:
        keep, first = [], False
        for inst in blk.instructions:
            if type(inst).__name__ == "InstLoadActFuncSet":
                si = getattr(inst, "sync_info", None)
                clean = si is None or (not si.on_wait and not si.on_update)
                if not first:
                    inst.act_func_set_id = nle
                    first = True
                    keep.append(inst)
                elif not clean:
                    inst.act_func_set_id = nle
                    keep.append(inst)
            else:
                keep.append(inst)
        blk.instructions[:] = keep
    return nc


def _prep(inputs):
    g1 = np.asarray(inputs["g1"], np.float32)
    b1 = np.asarray(inputs["b1"], np.float32)
    g2 = np.asarray(inputs["g2"], np.float32)
    b2 = np.asarray(inputs["b2"], np.float32)
    b_out = np.asarray(inputs["b_out"], np.float32)

    Tq, s_q = _ternary(inputs["W_qkv"])   # [3*inner, d]
    To, s_o = _ternary(inputs["W_out"])   # [dout, o]

    Wp = To * g2[None, :]                 # fold g2 (exact when g2 == 1)
    toT = np.ascontiguousarray(Wp.T)      # [o, dout]
    w1u = Wp.sum(axis=1).astype(np.float32)
    bias_total = (b2 @ To.T) * np.float32(s_o) + b_out

    LOG2E = 1.4426950408889634
    scale_exp = float(s_q * s_q * (DH ** -0.5))
    c = {
        "scale_exp": scale_exp,
        "fexp_a": float(scale_exp * LOG2E * 128.0),
        "fexp_b": float(16256.0 - 4.0),
        "inv_so2": float(1.0 / (s_o * s_o)),
        "eps_eff": float(EPS_LN / (s_q * s_q * s_o * s_o)),
        "need_g1": bool(not np.allclose(g1, 1.0)),
        "need_b1": bool(np.any(b1)),
        "need_bt": bool(np.any(bias_total)),
    }
    arrs = {
        "tqT": np.ascontiguousarray(Tq.T),
        "toT": toT,
        "w1u": w1u,
        "g1": g1, "b1": b1, "bt": bias_total,
    }
    return c, arrs


def _to_bf16(a):
    import ml_dtypes
    return np.asarray(a, np.float32).astype(ml_dtypes.bfloat16)


def _to_fp8(a):
    import ml_dtypes
    return np.asarray(a, np.float32).astype(ml_dtypes.float8_e4m3)


def kernel(**inputs) -> np.ndarray:
    global LAST_RESULTS
    x = np.asarray(inputs["x"], np.float32)
    assert x.shape == (B, N, D)
    c, arrs = _prep(inputs)

    key = tuple(sorted(c.items()))
    if key not in _CACHE:
        _CACHE[key] = _build(c)
    nc = _CACHE[key]

    base = {
        "tqT": _to_bf16(arrs["tqT"]),
        "toT": _to_bf16(arrs["toT"]),
        "w1u": arrs["w1u"].astype(np.float32),
    }
    if c["need_g1"]:
        base["g1v"] = arrs["g1"]
    if c["need_b1"]:
        base["b1v"] = arrs["b1"]
    if c["need_bt"]:
        base["btv"] = arrs["bt"].astype(np.float32)

    in_maps = [dict(base, x=np.ascontiguousarray(_to_bf16(x[i]))) for i in range(B)]
    res = run_bass_kernel_spmd(nc, in_maps, core_ids=list(range(B)), trace=TRACE)
    LAST_RESULTS = res
    out = np.stack([res.results[i]["y"] for i in range(B)], axis=0)
    return out.astype(np.float32)


def _pjrt_runner(nc, in_maps):
    """Build a jitted single-execution runner for a compiled Bass module on
    the 8 axon cores. Returns a 0-arg callable that runs + blocks."""
    import jax
    from jax.experimental.shard_map import shard_map
    from jax.sharding import Mesh, PartitionSpec, NamedSharding
    from concourse import bass2jax

    bass2jax.install_neuronx_cc_hook()
    partition_name = nc.partition_id_tensor.name if nc.partition_id_tensor else None
    in_names, out_names, out_avals, zero_outs = [], [], [], []
    for alloc in nc.m.functions[0].allocations:
        if not isinstance(alloc, mybir.MemoryLocationSet):
            continue
        name = alloc.memorylocations[0].name
        if alloc.kind == "ExternalInput":
            if name != partition_name:
                in_names.append(name)
        elif alloc.kind == "ExternalOutput":
            out_names.append(name)
            shape = tuple(alloc.tensor_shape)
            dtype = mybir.dt.np(alloc.dtype)
            out_avals.append(jax.core.ShapedArray(shape, dtype))
            zero_outs.append(np.zeros(shape, dtype))
    n_params = len(in_names)
    bind_names = list(in_names) + list(out_names)
    if partition_name is not None:
        bind_names.append(partition_name)

    def _body(*args):
        operands = list(args)
        pid = [bass2jax.partition_id_tensor()] if partition_name else []
        outs = bass2jax._bass_exec_p.bind(
            *(operands + pid),
            out_avals=tuple(out_avals),
            in_names=tuple(bind_names),
            out_names=tuple(out_names),
            lowering_input_output_aliases=(),
            sim_require_finite=True,
            sim_require_nnan=True,
            nc=nc,
        )
        return tuple(outs)

    devices = jax.devices()[:B]
    mesh = Mesh(np.asarray(devices), ("core",))
    spec = PartitionSpec("core")
    n_out = len(out_names)
    per_core = [[np.asarray(m[nm]) for nm in in_names] for m in in_maps]
    concat_in = [
        np.concatenate([per_core[cc][i] for cc in range(B)], axis=0)
        for i in range(n_params)
    ]
    concat_zeros = [
        np.zeros((B * z.shape[0], *z.shape[1:]), z.dtype) for z in zero_outs
    ]
    dev_args = [
        jax.device_put(a, NamedSharding(mesh, spec)) for a in concat_in + concat_zeros
    ]
    f = jax.jit(
        shard_map(
            _body, mesh=mesh,
            in_specs=(spec,) * (n_params + n_out),
            out_specs=(spec,) * n_out,
            check_rep=False,
        )
    )

    def run():
        jax.block_until_ready(f(*dev_args))

    run()  # compile + warm
    return run


def _bench_in_maps(inputs):
    x = np.asarray(inputs["x"], np.float32)
    c, arrs = _prep(inputs)
    base = {
        "tqT": _to_bf16(arrs["tqT"]),
        "toT": _to_bf16(arrs["toT"]),
        "w1u": arrs["w1u"].astype(np.float32),
    }
    if c["need_g1"]:
        base["g1v"] = arrs["g1"]
    if c["need_b1"]:
        base["b1v"] = arrs["b1"]
    if c["need_bt"]:
        base["btv"] = arrs["bt"].astype(np.float32)
    return c, [dict(base, x=np.ascontiguousarray(_to_bf16(x[i]))) for i in range(B)]


def bench_exec_ns_loop(inputs, loop_reps=129, reps=9):
    """Measure device exec time with a hardware For_i loop around the kernel
    body: one dispatch runs the body `loop_reps` times back-to-back on
    device, so exec = (T_loop - T_single) / (loop_reps - 1) with dispatch
    overhead cancelled and amortized over a large R."""
    import time as _time

    c, in_maps = _bench_in_maps(inputs)
    runners = {}
    for r in (1, loop_reps):
        cr = dict(c, loop_reps=r)
        key = tuple(sorted(cr.items()))
        if key not in _CACHE:
            _CACHE[key] = _build(cr)
        runners[r] = _pjrt_runner(_CACHE[key], in_maps)

    inner = 2  # calls per timing sample (averages dispatch jitter)
    samples = {1: [], loop_reps: []}
    for it in range(reps + 1):
        for r in (1, loop_reps) if it % 2 == 0 else (loop_reps, 1):
            t0 = _time.perf_counter()
            for _ in range(inner):
                runners[r]()
            samples[r].append((_time.perf_counter() - t0) / inner)
    # drop the first sample pair (warm-up drift), pair the rest
    diffs = sorted(
        (b - a) / (loop_reps - 1) * 1e9
        for a, b in zip(samples[1][1:], samples[loop_reps][1:])
    )
    exec_ns = diffs[len(diffs) // 2]
    times = {1: min(samples[1]), loop_reps: min(samples[loop_reps]),
             "diffs_us": [round(d / 1000, 1) for d in diffs]}
    return exec_ns, times


def bench_exec_ns_chain(inputs, iters=32, reps=7):
    """Measure per-execution device time by emitting `iters` sequential
    bass_exec custom calls inside ONE jitted program, data-chained by
    feeding each execution's y output back as the next x input (same
    shape/dtype). The device runs the kernels back-to-back in a single
    dispatch, so exec = (T_chain - T_single) / (iters - 1) cancels the
    per-dispatch axon overhead and its (large) jitter."""
    import time as _time
    import jax
    from jax.experimental.shard_map import shard_map
    from jax.sharding import Mesh, PartitionSpec, NamedSharding
    from concourse import bass2jax

    x = np.asarray(inputs["x"], np.float32)
    c, arrs = _prep(inputs)
    key = tuple(sorted(c.items()))
    if key not in _CACHE:
        _CACHE[key] = _build(c)
    nc = _CACHE[key]
    bass2jax.install_neuronx_cc_hook()

    base = {
        "tqT": _to_bf16(arrs["tqT"]),
        "toT": _to_bf16(arrs["toT"]),
        "w1u": arrs["w1u"].astype(np.float32),
    }
    if c["need_g1"]:
        base["g1v"] = arrs["g1"]
    if c["need_b1"]:
        base["b1v"] = arrs["b1"]
    if c["need_bt"]:
        base["btv"] = arrs["bt"].astype(np.float32)
    in_maps = [dict(base, x=np.ascontiguousarray(_to_bf16(x[i]))) for i in range(B)]

    partition_name = nc.partition_id_tensor.name if nc.partition_id_tensor else None
    in_names, out_names, out_avals, zero_outs = [], [], [], []
    for alloc in nc.m.functions[0].allocations:
        if not isinstance(alloc, mybir.MemoryLocationSet):
            continue
        name = alloc.memorylocations[0].name
        if alloc.kind == "ExternalInput":
            if name != partition_name:
                in_names.append(name)
        elif alloc.kind == "ExternalOutput":
            out_names.append(name)
            shape = tuple(alloc.tensor_shape)
            dtype = mybir.dt.np(alloc.dtype)
            out_avals.append(jax.core.ShapedArray(shape, dtype))
            zero_outs.append(np.zeros(shape, dtype))
    n_params = len(in_names)
    bind_names = list(in_names) + list(out_names)
    if partition_name is not None:
        bind_names.append(partition_name)
    xi = in_names.index("x")
    yi = out_names.index("y")

    def _make_body(k):
        def _body(*args):
            operands = list(args)
            pid = [bass2jax.partition_id_tensor()] if partition_name else []
            outs = None
            for _ in range(k):
                outs = bass2jax._bass_exec_p.bind(
                    *(operands + pid),
                    out_avals=tuple(out_avals),
                    in_names=tuple(bind_names),
                    out_names=tuple(out_names),
                    lowering_input_output_aliases=(),
                    sim_require_finite=True,
                    sim_require_nnan=True,
                    nc=nc,
                )
                operands = list(operands)
                operands[xi] = outs[yi]  # serialize: next x <- this y
            return tuple(outs)
        return _body

    devices = jax.devices()[:B]
    mesh = Mesh(np.asarray(devices), ("core",))
    spec = PartitionSpec("core")
    n_out = len(out_names)
    per_core = [[np.asarray(m[nm]) for nm in in_names] for m in in_maps]
    concat_in = [
        np.concatenate([per_core[cc][i] for cc in range(B)], axis=0)
        for i in range(n_params)
    ]
    concat_zeros = [
        np.zeros((B * z.shape[0], *z.shape[1:]), z.dtype) for z in zero_outs
    ]
    dev_args = [
        jax.device_put(a, NamedSharding(mesh, spec)) for a in concat_in + concat_zeros
    ]

    fs = {}
    for k in (1, iters):
        fs[k] = jax.jit(
            shard_map(
                _make_body(k), mesh=mesh,
                in_specs=(spec,) * (n_params + n_out),
                out_specs=(spec,) * n_out,
                check_rep=False,
            )
        )
        jax.block_until_ready(fs[k](*dev_args))  # compile + warm

    # alternate k=1 / k=iters samples so slow drift cancels in the pairing
    samples = {1: [], iters: []}
    for _ in range(reps):
        for k in (1, iters):
            t0 = _time.perf_counter()
            jax.block_until_ready(fs[k](*dev_args))
            samples[k].append(_time.perf_counter() - t0)
    diffs = sorted(
        (b - a) / (iters - 1) * 1e9
        for a, b in zip(samples[1], samples[iters])
    )
    exec_ns = diffs[len(diffs) // 2]  # median paired difference
    times = {1: min(samples[1]), iters: min(samples[iters]),
             "diffs_us": [round(d / 1000, 1) for d in diffs]}
    return exec_ns, times


def bench_exec_ns(inputs, iters=32, reps=5, body_reps=1):
    """Measure per-execution NEFF time by chaining `iters` sequential
    executions inside one jitted program (chained through the output
    buffers) and comparing against a 1-execution program."""
    import time as _time
    import jax
    from jax.experimental.shard_map import shard_map
    from jax.sharding import Mesh, PartitionSpec, NamedSharding
    from concourse import bass2jax, mybir as _mybir

    x = np.asarray(inputs["x"], np.float32)
    c, arrs = _prep(inputs)
    if body_reps != 1:
        c["body_reps"] = body_reps
    key = tuple(sorted(c.items()))
    if key not in _CACHE:
        _CACHE[key] = _build(c)
    nc = _CACHE[key]
    bass2jax.install_neuronx_cc_hook()

    base = {
        "tqT": _to_bf16(arrs["tqT"]),
        "toT": _to_bf16(arrs["toT"]),
        "w1u": arrs["w1u"].astype(np.float32),
    }
    if c["need_g1"]:
        base["g1v"] = arrs["g1"]
    if c["need_b1"]:
        base["b1v"] = arrs["b1"]
    if c["need_bt"]:
        base["btv"] = arrs["bt"].astype(np.float32)
    in_maps = [dict(base, x=np.ascontiguousarray(_to_bf16(x[i]))) for i in range(B)]

    partition_name = nc.partition_id_tensor.name if nc.partition_id_tensor else None
    in_names, out_names, out_avals, zero_outs = [], [], [], []
    for alloc in nc.m.functions[0].allocations:
        if not isinstance(alloc, mybir.MemoryLocationSet):
            continue
        name = alloc.memorylocations[0].name
        if alloc.kind == "ExternalInput":
            if name != partition_name:
                in_names.append(name)
        elif alloc.kind == "ExternalOutput":
            out_names.append(name)
            shape = tuple(alloc.tensor_shape)
            dtype = mybir.dt.np(alloc.dtype)
            out_avals.append(jax.core.ShapedArray(shape, dtype))
            zero_outs.append(np.zeros(shape, dtype))
    n_params = len(in_names)

    bind_names = list(in_names) + list(out_names)
    if partition_name is not None:
        bind_names.append(partition_name)

    def _body(*args):
        operands = list(args)
        pid = [bass2jax.partition_id_tensor()] if partition_name else []
        outs = bass2jax._bass_exec_p.bind(
            *(operands + pid),
            out_avals=tuple(out_avals),
            in_names=tuple(bind_names),
            out_names=tuple(out_names),
            lowering_input_output_aliases=(),
            sim_require_finite=True,
            sim_require_nnan=True,
            nc=nc,
        )
        return tuple(outs)

    devices = jax.devices()[:B]
    mesh = Mesh(np.asarray(devices), ("core",))
    spec = PartitionSpec("core")
    n_out = len(out_names)
    per_core = [[np.asarray(m[nm]) for nm in in_names] for m in in_maps]
    concat_in = [
        np.concatenate([per_core[cc][i] for cc in range(B)], axis=0)
        for i in range(n_params)
    ]
    concat_zeros = [
        np.zeros((B * z.shape[0], *z.shape[1:]), z.dtype) for z in zero_outs
    ]
    dev_args = [
        jax.device_put(a, NamedSharding(mesh, spec)) for a in concat_in + concat_zeros
    ]

    f = jax.jit(
        shard_map(
            _body, mesh=mesh,
            in_specs=(spec,) * (n_params + n_out),
            out_specs=(spec,) * n_out,
            check_rep=False,
        )
    )
    jax.block_until_ready(f(*dev_args))  # compile + warm

    times = {}
    for k in (1, iters):
        best = float("inf")
        for _ in range(reps):
            t0 = _time.perf_counter()
            r = None
            for _ in range(k):
                r = f(*dev_args)  # async dispatch; device executes in-order
            jax.block_until_ready(r)
            best = min(best, _time.perf_counter() - t0)
        times[k] = best
    exec_ns = (times[iters] - times[1]) / (iters - 1) * 1e9
    return exec_ns, times



# revision 46
# speedup vs baseline: 1.0653x; 1.0653x over previous
"""Trainium2 Bass kernel for nn_Attention_6794638262338.

Single-layer attention block with BitNet-style ternary-quantized projections:
    x -> LN1 -> qkv proj (ternary W) -> MHA softmax -> LN2 -> out proj (ternary W)

Strategy: pure data parallelism. batch=8, n_cores=8 -> one batch element per
core, no collectives. Each core runs an identical Bass/Tile program.

Math folds (host side):
  - ternary_quant(W) = T * s with T in {-1,0,1}: pass T in bf16 (exact), fold
    s_qkv^2 * DIM_HEAD^-0.5 into the exp() activation scale, fold s_qkv/s_out
    into the LN2 rsqrt epsilon/scale.
  - softmax denominator: out = (sum_m exp(s)*v) / colsum. colsum obtained free
    by appending a ones-column to v in the attn@v matmul (M=65); division done
    via DVE reciprocal + GpSimd partition_broadcast + DVE multiply.
  - LN2: mean/var via ones-matmul column sums of a^T, tiny PE transposes to get
    per-row stats, y = (z - mu*W1) * rsqrt-ish using host-precomputed
    W1 = rowsum of effective output weight.
"""

import numpy as np
from contextlib import ExitStack

import concourse.bass as bass
import concourse.mybir as mybir
import concourse.tile as tile
from concourse import bacc
from concourse.bass import ts, ds
from concourse.bass_utils import run_bass_kernel_spmd
from concourse.masks import make_identity

F32 = mybir.dt.float32
BF16 = mybir.dt.bfloat16
FP8 = mybir.dt.float8e4
DR = mybir.MatmulPerfMode.DoubleRow
AF = mybir.ActivationFunctionType
ALU = mybir.AluOpType

B, N, D = 8, 1024, 512
H, DH = 8, 64
INNER = H * DH  # 512
NT = N // 128   # 8 n-tiles
DC = D // 128   # 4 d-chunks
EPS_LN = 1e-5
EPS_Q = 1e-6

TRACE = False          # set by test.py to capture an NTFF profile
LAST_RESULTS = None    # BassKernelResults of the most recent run

_CACHE = {}


def _ternary(w):
    """Replicate reference ternary_quant in fp32; return (unit ternary, scale)."""
    w = np.asarray(w, np.float32)
    s = np.float32(np.mean(np.abs(w), dtype=np.float32))
    t = np.round(np.clip(w / (s + np.float32(EPS_Q)), -1.0, 1.0)).astype(np.float32)
    return t, float(s)


def _emit(ctx: ExitStack, tc: "tile.TileContext", io: dict, c: dict, sfx: str = ""):
    nc = tc.nc
    dbg = c.get("debug", False)
    loop_reps = c.get("loop_reps", 0)

    def dump(name, ap):
        if dbg:
            d = nc.dram_tensor(f"dbg_{name}{sfx}", list(ap.shape), ap.dtype, kind="ExternalOutput").ap()
            nc.sync.dma_start(out=d, in_=ap)
    x, tqT, toT, w1u, y = io["x"], io["tqT"], io["toT"], io["w1u"], io["y"]

    need_g1 = c["need_g1"]
    need_b1 = c["need_b1"]
    need_bt = c["need_bt"]

    # ---------------- pools ----------------
    const_p = ctx.enter_context(tc.tile_pool(name="const" + sfx, bufs=1))
    xp = ctx.enter_context(tc.tile_pool(name="xp" + sfx, bufs=3))
    lnp = ctx.enter_context(tc.tile_pool(name="lnp" + sfx, bufs=4))
    xlnp = ctx.enter_context(tc.tile_pool(name="xlnp" + sfx, bufs=3))
    big = ctx.enter_context(tc.tile_pool(name="big" + sfx, bufs=1))
    attp = ctx.enter_context(tc.tile_pool(name="attp" + sfx, bufs=2))
    smp = ctx.enter_context(tc.tile_pool(name="smp" + sfx, bufs=3))
    outp = ctx.enter_context(tc.tile_pool(name="outp" + sfx, bufs=2))
    # PSUM budget: 8 banks = ps_s ([128,1024] x2 = 4) + ps_o ([65,512] x2 = 2)
    #              + ps_m ([128,512] x2 = 2)
    ps_s = ctx.enter_context(tc.tile_pool(name="ps_s" + sfx, bufs=2, space="PSUM"))
    ps_o = ctx.enter_context(tc.tile_pool(name="ps_o" + sfx, bufs=2, space="PSUM"))
    ps_m = ctx.enter_context(tc.tile_pool(name="ps_m" + sfx, bufs=2, space="PSUM"))

    # ---------------- constants ----------------
    ident = const_p.tile([128, 128], BF16)
    make_identity(nc, ident)
    # stats column: 1/INNER folded in, so the s1/s2 ones-matmuls produce
    # mu and E[a^2] directly (drops two ACT hops per LN2 group)
    ones128 = const_p.tile([128, 1], BF16)
    nc.vector.memset(ones128, 1.0 / INNER)
    eps1 = const_p.tile([128, 1], F32)
    nc.vector.memset(eps1, float(EPS_LN))
    eps2 = const_p.tile([128, 1], F32)
    nc.vector.memset(eps2, c["eps_eff"])
    # warm the ln/exp activation table while the first x tile is in flight
    warm = const_p.tile([128, 1], F32)
    nc.scalar.activation(warm, eps1, AF.Ln, bias=eps1)
    nc.scalar.activation(warm, warm, AF.Exp, scale=-0.5)

    # weight loads go on the GpSimd DMA queue so the x tiles own the SP
    # queue from t=0 (they gate the LN1->transpose critical path)
    # qkv unit-ternary weights, transposed: [d, 3*inner] -> sbuf [128, DC, 3*inner]
    tq_sb = const_p.tile([128, DC, 3 * INNER], BF16)
    nc.gpsimd.dma_start(out=tq_sb, in_=tqT.rearrange("(c p) o -> p c o", p=128))
    # out-proj unit weights (g2 folded), transposed: [o, dout] -> [128, DC, dout]
    toT_sb = const_p.tile([128, DC, INNER], BF16)
    nc.gpsimd.dma_start(out=toT_sb, in_=toT.rearrange("(c p) o -> p c o", p=128))
    # W1 rowsums broadcast across partitions
    w1b = const_p.tile([128, INNER], F32)
    nc.gpsimd.dma_start(
        out=w1b,
        in_=bass.AP(tensor=w1u.tensor, offset=w1u.offset, ap=[[0, 128]] + list(w1u.ap)),
    )
    if need_g1:
        g1_ap = io["g1v"]
        g1b = const_p.tile([128, D], F32)
        nc.gpsimd.dma_start(
            out=g1b,
            in_=bass.AP(tensor=g1_ap.tensor, offset=g1_ap.offset, ap=[[0, 128]] + list(g1_ap.ap)),
        )
    if need_b1:
        b1_ap = io["b1v"]
        b1b = const_p.tile([128, D], F32)
        nc.gpsimd.dma_start(
            out=b1b,
            in_=bass.AP(tensor=b1_ap.tensor, offset=b1_ap.offset, ap=[[0, 128]] + list(b1_ap.ap)),
        )
    if need_bt:
        bt_ap = io["btv"]
        btb = const_p.tile([128, INNER], F32)
        nc.gpsimd.dma_start(
            out=btb,
            in_=bass.AP(tensor=bt_ap.tensor, offset=bt_ap.offset, ap=[[0, 128]] + list(bt_ap.ap)),
        )

    scale_exp = c["scale_exp"]

    def body():
        # ---------------- persistent big tensors ----------------
        # xln^T: [d, n] bf16, split in two n-half tiles [128, DC, 512] so the
        # first qkv matmuls start after only half of Phase A
        xlnTh = [
            big.tile([128, DC, 512], BF16, name=f"xlnTh{i}", tag=f"xlnTh{i}")
            for i in range(2)
        ]
        # q^T, k^T head-major: [o, n] as [128, DC, N] (o = otile*128 + p)
        qT = big.tile([128, DC, N], BF16)
        kT = big.tile([128, DC, N], BF16)
        # v row-major with ones column: [128, mt, h, 65] (m = mt*128 + p)
        v_sb = big.tile([128, NT, H, DH + 1], BF16)
        nc.vector.memset(v_sb[:, :, :, DH : DH + 1], 1.0)
        # pair-stacked divided attention out: partition 0:64 = head 2p,
        # 64:128 = head 2p+1 (DVE cross-partition writes)
        aT2 = big.tile([128, DC, N], BF16)
        # squares of aT2 for the LN2 sum-of-squares (filled by GpSimd)
        sq_sb = big.tile([128, DC, N], BF16)

        def emit_qk(ot, nns=(0, 1)):
            # q, k head-major: psum[o_tile, n] = sum_dc Tq[:,dc,ot].T @ xlnT[:,dc,n]
            # k before q per n-half: the first scores matmul needs (kT nn0,
            # qT nn0) only. Copies run on ACT (Copy activation) to keep DVE
            # free for the fast-exp tiles + divides.
            for nn in nns:
                for sec, dst in ((1, kT), (0, qT)):
                    pq = ps_m.tile([128, 512], F32, name="pq", tag="mm")
                    for dc in range(DC):
                        nc.tensor.matmul(
                            pq,
                            lhsT=tq_sb[:, dc, ds(sec * INNER + ot * 128, 128)],
                            rhs=xlnTh[nn][:, dc, :],
                            start=(dc == 0), stop=(dc == DC - 1),
                        )
                    if c.get("qk_act", True):
                        nc.scalar.activation(out=dst[:, ot, ts(nn, 512)], in_=pq, func=AF.Copy)
                    else:
                        nc.vector.tensor_copy(out=dst[:, ot, ts(nn, 512)], in_=pq)


        # ================ Phase A: load x, LN1, transpose ================
        for nt in range(NT):
            # x arrives bf16 (host-converted): halves the input DMA bytes
            xt = xp.tile([128, D], BF16, name="xt", tag="xt")
            nc.sync.dma_start(out=xt, in_=x[ts(nt, 128), :])
            st6 = lnp.tile([128, 6], F32, name="st6", tag="st6")
            nc.vector.bn_stats(st6, xt)
            mv = lnp.tile([128, 2], F32, name="mv", tag="mv")
            nc.vector.bn_aggr(mv, st6)
            # rstd = exp(-0.5*ln(var+eps)) — keeps ACT on the ln/exp table set
            # (same set the attention exp uses; avoids sqrt-set thrashing)
            sd = lnp.tile([128, 1], F32, name="sd", tag="sd")
            nc.scalar.activation(sd, mv[:, 1:2], AF.Ln, bias=eps1)
            rs = lnp.tile([128, 1], F32, name="rs", tag="rs")
            nc.scalar.activation(rs, sd, AF.Exp, scale=-0.5)
            xl = xlnp.tile([128, D], BF16, name="xl", tag="xl")
            if need_g1 or need_b1:
                xlf = xlnp.tile([128, D], F32, name="xlf", tag="xlf")
                nc.vector.tensor_scalar(
                    out=xlf, in0=xt, scalar1=mv[:, 0:1], scalar2=rs,
                    op0=ALU.subtract, op1=ALU.mult,
                )
                if need_g1:
                    nc.vector.tensor_mul(xlf, xlf, g1b)
                if need_b1:
                    nc.vector.tensor_add(xlf, xlf, b1b)
                nc.vector.tensor_copy(xl, xlf)
            else:
                # (x - mu)*rs on ACT (idle in Phase A; DVE is the pacer):
                # Identity activation with per-partition scale rs and bias
                # -mu*rs
                nrsmu = lnp.tile([128, 1], F32, name="nrsmu", tag="nrsmu")
                nc.vector.tensor_scalar(
                    out=nrsmu, in0=mv[:, 0:1], scalar1=rs, scalar2=-1.0,
                    op0=ALU.mult, op1=ALU.mult,
                )
                nc.scalar.activation(out=xl, in_=xt, func=AF.Identity, scale=rs, bias=nrsmu)
            # transpose via matmul with identity: out = xl_slice.T. All four
            # d-chunks land in one psum tile -> one strided copy into xlnT.
            # Copies for the second half go to GpSimd: DVE is the Phase A
            # pacer and the weight DMAs have drained off the Pool queue by
            # then.
            pt = ps_m.tile([128, DC, 128], F32, name="pt", tag="mm")
            for dc in range(DC):
                nc.tensor.matmul(
                    pt[:, dc, :], lhsT=xl[:, ts(dc, 128)], rhs=ident, start=True, stop=True
                )
            nc.vector.tensor_copy(out=xlnTh[nt // 4][:, :, ts(nt % 4, 128)], in_=pt)
            if nt == 3:
                emit_qk(0, nns=(0,))  # first n-half of q/k as soon as it exists

        # ================ Phase B+C interleaved: qkv otiles feed attention
        # head-pairs as soon as their q/k tile is ready, so ACT starts exp()
        # early and stays the pacer without idle lead-in. ================
        def emit_v(mts):
            # v row-major: psum[m_tile, o] = sum_dc xlnT[:,dc,mt].T @ Tq_v[:,dc,:]
            for mt in mts:
                pv = ps_m.tile([128, 512], F32, name="pv", tag="mm")
                for dc in range(DC):
                    nc.tensor.matmul(
                        pv,
                        lhsT=xlnTh[mt // 4][:, dc, ts(mt % 4, 128)],
                        rhs=tq_sb[:, dc, ds(2 * INNER, INNER)],
                        start=(dc == 0), stop=(dc == DC - 1),
                    )
                # strided copy into per-head layout [128, h, 64]
                nc.vector.tensor_copy(
                    out=v_sb[:, mt, :, 0:DH],
                    in_=pv.rearrange("p (h d) -> p h d", h=H),
                )

        # ~1/3 of the exp tiles run on DVE as a Schraudolph bf16-bit-trick
        # (bits = round(s*scale*log2e*128 + 16256-shift) as int16, bitcast to
        # bf16 ~= exp(s*scale) with ~3% sawtooth err that mostly cancels in
        # softmax). This splits the exp wall (the mid-kernel pacer) between
        # ACT and DVE. Tail-critical (mt=7) tiles stay on ACT.
        fexp_a, fexp_b = c["fexp_a"], c["fexp_b"]
        I16 = mybir.dt.int16

        _fexp_sets = {
            0: {0: (), 1: ()},
            16: {0: (1, 4), 1: (2, 5)},
            24: {0: (1, 3, 5), 1: (2, 4, 6)},
            32: {0: (1, 3, 5, 6), 1: (0, 2, 4, 5)},
            40: {0: (0, 1, 3, 5, 6), 1: (0, 2, 3, 4, 5)},
        }

        def fexp_on_dve(p, sub, mt):
            if not c.get("fexp", True):
                return False
            return mt in _fexp_sets[c.get("fexp_n", 24)][sub]

        def emit_scores_pair(p, interleave=None):
            """Scores+exp for heads 2p (partitions 0:64) and 2p+1 (64:128).
            atn is one tile per (sub, mt) so downstream attn@v matmuls only
            wait on the exps they actually read. `interleave(j)` is called
            after exp(mt=j+3) to slot tail-pair attn@v matmuls between
            score matmuls."""
            atns = {0: {}, 1: {}}
            for mt in range(NT):
                pss = [
                    ps_s.tile([128, N], F32, name="pssa", tag="s"),
                    ps_s.tile([128, N], F32, name="pssb", tag="s"),
                ]
                # sub-major order: each sub's two n-half matmuls are
                # adjacent, so its exp fires one matmul earlier
                for sub in range(2):
                    for nn in range(2):
                        base = sub * 64
                        nc.tensor.matmul(
                            pss[sub][:, ts(nn, 512)],
                            lhsT=kT[ds(base, 64), p, ts(mt, 128)],
                            rhs=qT[ds(base, 64), p, ts(nn, 512)],
                            start=True, stop=True,
                        )
                for sub in range(2):
                    if fexp_on_dve(p, sub, mt):
                        ai = attp.tile([128, N], I16, name=f"atn{sub}_{mt}", tag=f"atn{sub}_{mt}")
                        nc.vector.tensor_scalar(
                            out=ai, in0=pss[sub], scalar1=fexp_a, scalar2=fexp_b,
                            op0=ALU.mult, op1=ALU.add,
                        )
                        a = ai.bitcast(BF16)
                    else:
                        a = attp.tile([128, N], BF16, name=f"atn{sub}_{mt}", tag=f"atn{sub}_{mt}")
                        nc.scalar.activation(out=a, in_=pss[sub], func=AF.Exp, scale=scale_exp)
                    atns[sub][mt] = a
                if interleave is not None and mt >= 3:
                    interleave(mt - 3, atns)
            return atns

        def divide(h, po2):
            # divide chain straight off PSUM: reciprocal of the colsum row
            # (DVE reads psum p64, writes a partition-0 staging row — engines
            # CAN shift partitions, HW-verified) -> GpSimd partition_broadcast
            # (source must be in partitions 0..15: Q7 core 0 does the read)
            # -> multiply straight into aT2 rows 0:64 / 64:128 (cross-
            # partition DVE write kills the old odd-head remap DMA).
            rc0 = smp.tile([1, 2, 512], F32, name="rc0", tag="rc0")
            for nn in range(2):
                nc.vector.reciprocal(rc0[:, nn, :], po2[nn][64:65, :])
            rbt = smp.tile([64, 2, 512], F32, name="rbt", tag="rbt")
            if c.get("pbcast", False):
                nc.gpsimd.partition_broadcast(rbt, rc0, channels=64)
            else:
                bc_dram = nc.dram_tensor(f"cs_scratch{h}" + sfx, [2, 512], F32).ap()
                nc.sync.dma_start(out=bc_dram, in_=rc0)
                nc.sync.dma_start(
                    out=rbt,
                    in_=bass.AP(tensor=bc_dram.tensor, offset=bc_dram.offset,
                                ap=[[0, 64]] + list(bc_dram.ap)),
                )
            for nn in range(2):
                nc.vector.tensor_tensor(
                    out=aT2[ds(64 * (h % 2), 64), h // 2, ts(nn, 512)],
                    in0=po2[nn][0:64, :], in1=rbt[:, nn, :], op=ALU.mult,
                )

        def emit_out(h, atn, pool=None, tag="po"):
            po2 = [
                (pool or ps_o).tile([65, 512], F32, name=f"po{nn}", tag=tag)
                for nn in range(2)
            ]
            for mt in range(NT):
                for nn in range(2):
                    nc.tensor.matmul(
                        po2[nn],
                        lhsT=v_sb[:, mt, h, :],
                        rhs=atn[mt][:, ts(nn, 512)],
                        start=(mt == 0), stop=(mt == NT - 1),
                    )
            divide(h, po2)

        # driver: scores-pair 0 starts as soon as its q/k tile exists (ACT
        # starts exp'ing early); v and the next pair's q/k are emitted behind
        # the current pair's scores so PE fills its exp-wait slack with them;
        # out-matmuls run one pair behind. Squares for the LN2 sum-of-squares
        # run on idle GpSimd as chunks finish (last chunk on DVE: tail-critical).
        emit_qk(0, nns=(1,))  # nn0 was emitted inside Phase A at nt==3
        prev = emit_scores_pair(0)
        emit_v(range(0, 8))
        emit_qk(1)
        for pair in range(1, 3):
            atns = emit_scores_pair(pair)
            emit_qk(pair + 1)
            pp = pair - 1
            emit_out(2 * pp, prev[0])
            emit_out(2 * pp + 1, prev[1])
            nc.gpsimd.tensor_mul(sq_sb[:, pp, :], aT2[:, pp, :], aT2[:, pp, :])
            prev = atns

        # pair 3: head 6's attn@v accumulation is interleaved into the
        # scores loop three exp-steps behind (borrowing the ps_m slots,
        # idle until phase D), so only its last three accumulation steps
        # trail the final exp. Pair 2's out matmuls + divides drain from a
        # work queue a few per step so the PE load stays level and ACT
        # never starves. Head 7 runs after the loop as usual.
        po6 = [ps_m.tile([65, 512], F32, name=f"po6_{nn}", tag="mm") for nn in range(2)]

        po45 = {}
        pending = []

        def _alloc45(h):
            po45[h] = [
                ps_o.tile([65, 512], F32, name=f"po{nn}", tag="po") for nn in range(2)
            ]

        def _mm45(h, atn, mt):
            for nn in range(2):
                nc.tensor.matmul(
                    po45[h][nn],
                    lhsT=v_sb[:, mt, h, :],
                    rhs=atn[mt][:, ts(nn, 512)],
                    start=(mt == 0), stop=(mt == NT - 1),
                )

        for _h, _sub in ((4, 0), (5, 1)):
            pending.append((lambda h=_h: _alloc45(h)))
            for _mt in range(NT):
                pending.append(lambda h=_h, s=_sub, mt=_mt: _mm45(h, prev[s], mt))
            pending.append(lambda h=_h: divide(h, po45[h]))
        pending.append(lambda: nc.gpsimd.tensor_mul(sq_sb[:, 2, :], aT2[:, 2, :], aT2[:, 2, :]))

        def tail_out(j, atns3):
            for nn in range(2):
                nc.tensor.matmul(
                    po6[nn], lhsT=v_sb[:, j, 6, :], rhs=atns3[0][j][:, ts(nn, 512)],
                    start=(j == 0), stop=(j == NT - 1),
                )
            for _ in range(5):
                if pending:
                    pending.pop(0)()

        atns3 = emit_scores_pair(3, interleave=tail_out)
        while pending:
            pending.pop(0)()
        for j in range(5, 8):
            tail_out(j, atns3)
        # head 7 attn@v on ps_s slots (free after the last exps) so it does
        # not wait for head 5's divide to release a ps_o slot
        po7 = [ps_s.tile([65, 512], F32, name=f"po7_{nn}", tag="s") for nn in range(2)]
        for mt in range(NT):
            for nn in range(2):
                nc.tensor.matmul(
                    po7[nn], lhsT=v_sb[:, mt, 7, :], rhs=atns3[1][mt][:, ts(nn, 512)],
                    start=(mt == 0), stop=(mt == NT - 1),
                )
        divide(6, po6)
        divide(7, po7)
        nc.vector.tensor_mul(sq_sb[:, 3, :], aT2[:, 3, :], aT2[:, 3, :])

        dump("qT", qT)
        dump("kT", kT)
        dump("v", v_sb)
        dump("aT2", aT2)

        # ================ Phase D: LN2 stats + output projection ================
        # Four groups of 2 n-tiles. j=0's z goes to a [128, 513] ps_s tile
        # whose 513th column (toT_sb col 512 = 1/INNER) IS the s1 mean —
        # the s1 stats matmuls ride the projection for free. j=1 stays
        # [128, 512] on ps_m (1 bank) + explicit s1 matmuls, preserving the
        # 4-deep pz pipeline. The y ops avoid ACT entirely (Pool + DVE),
        # keeping ACT free for the exp wall.
        for g in range(4):
            # st[:, 0, j] = s1 (sum_o a), st[:, 1, j] = s2 (sum_o a^2)
            st = ps_o.tile([128, 2, 2], F32, name=f"st{g}", tag="po")
            pzs = []
            zsts = []
            for j in range(2):
                nt = 2 * g + j
                pz = (
                    ps_s.tile([128, INNER], F32, name="pz", tag="s")
                    if j == 0
                    else ps_m.tile([128, INNER], F32, name="pz", tag="mm")
                )
                pzs.append(pz)
                for ch in range(DC):
                    nc.tensor.matmul(
                        pz, lhsT=aT2[:, ch, ts(nt, 128)], rhs=toT_sb[:, ch, :],
                        start=(ch == 0), stop=(ch == DC - 1),
                    )
                    nc.tensor.matmul(
                        st[:, 0, j : j + 1], lhsT=aT2[:, ch, ts(nt, 128)], rhs=ones128,
                        start=(ch == 0), stop=(ch == DC - 1),
                    )
                for ch in range(DC):
                    nc.tensor.matmul(
                        st[:, 1, j : j + 1], lhsT=sq_sb[:, ch, ts(nt, 128)], rhs=ones128,
                        start=(ch == 0), stop=(ch == DC - 1),
                    )
                if j == 1:
                    # DVE-path tile stages z to SBUF; the ACT-path tile (j=0)
                    # reads its z PSUM directly via Identity
                    zst = outp.tile([128, INNER], BF16, name="zst", tag="zst", bufs=2)
                    nc.vector.tensor_copy(zst, pz)
                    zsts.append(zst)

            # the 1/INNER fold makes mu and E[a^2] direct; var = E[a^2]-mu^2;
            # r2 = s_o / sqrt(var + eps_eff). mu lifted to SBUF right away so
            # psum slots free early.
            muc = lnp.tile([128, 2], F32, name=f"muc{g}", tag="muc", bufs=2)
            nc.vector.tensor_copy(muc, st[:, 0, :])
            musq = lnp.tile([128, 2], F32, name=f"musq{g}", tag="musq", bufs=2)
            nc.vector.tensor_mul(musq, muc, muc)
            var = lnp.tile([128, 2], F32, name=f"var{g}", tag="var", bufs=2)
            nc.vector.tensor_sub(var, st[:, 1, :], musq)
            sd2 = lnp.tile([128, 2], F32, name=f"sd2{g}", tag="sd2", bufs=2)
            nc.scalar.activation(sd2, var, AF.Ln, bias=eps2, scale=c["inv_so2"])
            r2 = lnp.tile([128, 2], F32, name=f"r2{g}", tag="r2", bufs=2)
            nc.scalar.activation(r2, sd2, AF.Exp, scale=-0.5)
            r2n = lnp.tile([128, 2], F32, name=f"r2n{g}", tag="r2n", bufs=2)
            nc.vector.tensor_scalar_mul(r2n, r2, -1.0)
            # nmur2 = -mu*r2 for the rank-1 W1 term
            nmur2 = lnp.tile([128, 2], F32, name=f"nmur2{g}", tag="nmur2", bufs=2)
            nc.vector.tensor_mul(nmur2, muc, r2n)

            # y = (z - mu*W1) * r2 (+ bias_total)
            # j=0 (ACT path): y = Identity(z*r2) + Identity(W1*(-mu*r2)),
            #   summed on GpSimd — keeps the tail off DVE.
            # j=1 (DVE path): u = (W1*mu) - z ; y = u*(-r2)
            for j in range(2):
                nt = 2 * g + j
                # y is written bf16 (halves the output DMA bytes); the last
                # op of each path writes the bf16 tile directly
                yt = outp.tile([128, INNER], BF16, name="yt", tag="yt")
                if j == 0:
                    t1 = outp.tile([128, INNER], F32, name="t1", tag="t1", bufs=2)
                    nc.scalar.activation(out=t1, in_=pzs[0], func=AF.Identity,
                                         scale=r2[:, 0:1])
                    t2 = outp.tile([128, INNER], F32, name="t2", tag="t2", bufs=2)
                    nc.scalar.activation(out=t2, in_=w1b, func=AF.Identity,
                                         scale=nmur2[:, 0:1])
                    if need_bt:
                        ytf = outp.tile([128, INNER], F32, name="ytf", tag="ytf")
                        nc.gpsimd.tensor_add(ytf, t1, t2)
                        nc.gpsimd.tensor_add(yt, ytf, btb)
                    else:
                        nc.gpsimd.tensor_add(yt, t1, t2)
                else:
                    ut = outp.tile([128, INNER], F32, name="ut", tag="ut")
                    nc.vector.scalar_tensor_tensor(
                        out=ut, in0=w1b, scalar=muc[:, 1:2], in1=zsts[0],
                        op0=ALU.mult, op1=ALU.subtract,
                    )
                    if need_bt:
                        nc.vector.tensor_scalar_mul(ut, ut, r2n[:, 1:2])
                        nc.vector.tensor_add(yt, ut, btb)
                    else:
                        nc.vector.tensor_scalar_mul(yt, ut, r2n[:, 1:2])
                nc.sync.dma_start(out=y[ts(nt, 128), :], in_=yt)

    if loop_reps:
        with tc.For_i(0, loop_reps):
            body()
    else:
        body()


def _build(c: dict):
    nc = bacc.Bacc("TRN2", target_bir_lowering=False, debug=False, num_devices=B)
    io = {
        "x": nc.dram_tensor("x", [N, D], BF16, kind="ExternalInput").ap(),
        "tqT": nc.dram_tensor("tqT", [D, 3 * INNER], BF16, kind="ExternalInput").ap(),
        "toT": nc.dram_tensor("toT", [INNER, INNER], BF16, kind="ExternalInput").ap(),
        "w1u": nc.dram_tensor("w1u", [INNER], F32, kind="ExternalInput").ap(),
        "y": nc.dram_tensor("y", [N, D], BF16, kind="ExternalOutput").ap(),
    }
    if c["need_g1"]:
        io["g1v"] = nc.dram_tensor("g1v", [D], F32, kind="ExternalInput").ap()
    if c["need_b1"]:
        io["b1v"] = nc.dram_tensor("b1v", [D], F32, kind="ExternalInput").ap()
    if c["need_bt"]:
        io["btv"] = nc.dram_tensor("btv", [INNER], F32, kind="ExternalInput").ap()
    reps = c.get("body_reps", 1)
    with tile.TileContext(nc) as tc:
        for r in range(reps):
            with ExitStack() as ctx:
                _emit(ctx, tc, io, c, sfx="" if r == 0 else f"_r{r}")

    nc.compile()

    # The act-table-load pass greedily picks the first set containing each
    # function, thrashing between `natural_log` (Ln) and `exp_and_others`
    # (Exp) on every rstd computation (18 reloads @ ~1.3-2.7us each). All
    # activation funcs this kernel uses (Ln, Exp, Copy, Identity) live
    # together in `natural_log_exp_and_others`, so rewrite the first load to
    # that set and drop the rest.
    from concourse.hw_specs import get_activation_tables
    tset = list(get_activation_tables(nc.m.arch).keys())
    nle = tset.index("natural_log_exp_and_others")
    for blk in nc.main_func.blocks:
        keep, first = [], False
        for inst in blk.instructions:
            if type(inst).__name__ == "InstLoadActFuncSet":
                si = getattr(inst, "sync_info", None)
                clean = si is None or (not si.on_wait and not si.on_update)
                if not first:
                    inst.act_func_set_id = nle
                    first = True
                    keep.append(inst)
                elif not clean:
                    inst.act_func_set_id = nle
                    keep.append(inst)
            else:
                keep.append(inst)
        blk.instructions[:] = keep
    return nc


def _prep(inputs):
    g1 = np.asarray(inputs["g1"], np.float32)
    b1 = np.asarray(inputs["b1"], np.float32)
    g2 = np.asarray(inputs["g2"], np.float32)
    b2 = np.asarray(inputs["b2"], np.float32)
    b_out = np.asarray(inputs["b_out"], np.float32)

    Tq, s_q = _ternary(inputs["W_qkv"])   # [3*inner, d]
    To, s_o = _ternary(inputs["W_out"])   # [dout, o]

    Wp = To * g2[None, :]                 # fold g2 (exact when g2 == 1)
    toT = np.ascontiguousarray(Wp.T)      # [o, dout]
    w1u = Wp.sum(axis=1).astype(np.float32)
    bias_total = (b2 @ To.T) * np.float32(s_o) + b_out

    LOG2E = 1.4426950408889634
    scale_exp = float(s_q * s_q * (DH ** -0.5))
    c = {
        "scale_exp": scale_exp,
        "fexp_a": float(scale_exp * LOG2E * 128.0),
        "fexp_b": float(16256.0 - 4.0),
        "inv_so2": float(1.0 / (s_o * s_o)),
        "eps_eff": float(EPS_LN / (s_q * s_q * s_o * s_o)),
        "need_g1": bool(not np.allclose(g1, 1.0)),
        "need_b1": bool(np.any(b1)),
        "need_bt": bool(np.any(bias_total)),
    }
    arrs = {
        "tqT": np.ascontiguousarray(Tq.T),
        "toT": toT,
        "w1u": w1u,
        "g1": g1, "b1": b1, "bt": bias_total,
    }
    return c, arrs


def _to_bf16(a):
    import ml_dtypes
    return np.asarray(a, np.float32).astype(ml_dtypes.bfloat16)


def _to_fp8(a):
    import ml_dtypes
    return np.asarray(a, np.float32).astype(ml_dtypes.float8_e4m3)


def kernel(**inputs) -> np.ndarray:
    global LAST_RESULTS
    x = np.asarray(inputs["x"], np.float32)
    assert x.shape == (B, N, D)
    c, arrs = _prep(inputs)

    key = tuple(sorted(c.items()))
    if key not in _CACHE:
        _CACHE[key] = _build(c)
    nc = _CACHE[key]

    base = {
        "tqT": _to_bf16(arrs["tqT"]),
        "toT": _to_bf16(arrs["toT"]),
        "w1u": arrs["w1u"].astype(np.float32),
    }
    if c["need_g1"]:
        base["g1v"] = arrs["g1"]
    if c["need_b1"]:
        base["b1v"] = arrs["b1"]
    if c["need_bt"]:
        base["btv"] = arrs["bt"].astype(np.float32)

    in_maps = [dict(base, x=np.ascontiguousarray(_to_bf16(x[i]))) for i in range(B)]
    res = run_bass_kernel_spmd(nc, in_maps, core_ids=list(range(B)), trace=TRACE)
    LAST_RESULTS = res
    out = np.stack([res.results[i]["y"] for i in range(B)], axis=0)
    return out.astype(np.float32)


def _pjrt_runner(nc, in_maps):
    """Build a jitted single-execution runner for a compiled Bass module on
    the 8 axon cores. Returns a 0-arg callable that runs + blocks."""
    import jax
    from jax.experimental.shard_map import shard_map
    from jax.sharding import Mesh, PartitionSpec, NamedSharding
    from concourse import bass2jax

    bass2jax.install_neuronx_cc_hook()
    partition_name = nc.partition_id_tensor.name if nc.partition_id_tensor else None
    in_names, out_names, out_avals, zero_outs = [], [], [], []
    for alloc in nc.m.functions[0].allocations:
        if not isinstance(alloc, mybir.MemoryLocationSet):
            continue
        name = alloc.memorylocations[0].name
        if alloc.kind == "ExternalInput":
            if name != partition_name:
                in_names.append(name)
        elif alloc.kind == "ExternalOutput":
            out_names.append(name)
            shape = tuple(alloc.tensor_shape)
            dtype = mybir.dt.np(alloc.dtype)
            out_avals.append(jax.core.ShapedArray(shape, dtype))
            zero_outs.append(np.zeros(shape, dtype))
    n_params = len(in_names)
    bind_names = list(in_names) + list(out_names)
    if partition_name is not None:
        bind_names.append(partition_name)

    def _body(*args):
        operands = list(args)
        pid = [bass2jax.partition_id_tensor()] if partition_name else []
        outs = bass2jax._bass_exec_p.bind(
            *(operands + pid),
            out_avals=tuple(out_avals),
            in_names=tuple(bind_names),
            out_names=tuple(out_names),
            lowering_input_output_aliases=(),
            sim_require_finite=True,
            sim_require_nnan=True,
            nc=nc,
        )
        return tuple(outs)

    devices = jax.devices()[:B]
    mesh = Mesh(np.asarray(devices), ("core",))
    spec = PartitionSpec("core")
    n_out = len(out_names)
    per_core = [[np.asarray(m[nm]) for nm in in_names] for m in in_maps]
    concat_in = [
        np.concatenate([per_core[cc][i] for cc in range(B)], axis=0)
        for i in range(n_params)
    ]
    concat_zeros = [
        np.zeros((B * z.shape[0], *z.shape[1:]), z.dtype) for z in zero_outs
    ]
    dev_args = [
        jax.device_put(a, NamedSharding(mesh, spec)) for a in concat_in + concat_zeros
    ]
    f = jax.jit(
        shard_map(
            _body, mesh=mesh,
            in_specs=(spec,) * (n_params + n_out),
            out_specs=(spec,) * n_out,
            check_rep=False,
        )
    )

    def run():
        jax.block_until_ready(f(*dev_args))

    run()  # compile + warm
    return run


def _bench_in_maps(inputs):
    x = np.asarray(inputs["x"], np.float32)
    c, arrs = _prep(inputs)
    base = {
        "tqT": _to_bf16(arrs["tqT"]),
        "toT": _to_bf16(arrs["toT"]),
        "w1u": arrs["w1u"].astype(np.float32),
    }
    if c["need_g1"]:
        base["g1v"] = arrs["g1"]
    if c["need_b1"]:
        base["b1v"] = arrs["b1"]
    if c["need_bt"]:
        base["btv"] = arrs["bt"].astype(np.float32)
    return c, [dict(base, x=np.ascontiguousarray(_to_bf16(x[i]))) for i in range(B)]


def bench_exec_ns_loop(inputs, loop_reps=129, reps=9):
    """Measure device exec time with a hardware For_i loop around the kernel
    body: one dispatch runs the body `loop_reps` times back-to-back on
    device, so exec = (T_loop - T_single) / (loop_reps - 1) with dispatch
    overhead cancelled and amortized over a large R."""
    import time as _time

    c, in_maps = _bench_in_maps(inputs)
    runners = {}
    for r in (1, loop_reps):
        cr = dict(c, loop_reps=r)
        key = tuple(sorted(cr.items()))
        if key not in _CACHE:
            _CACHE[key] = _build(cr)
        runners[r] = _pjrt_runner(_CACHE[key], in_maps)

    inner = 2  # calls per timing sample (averages dispatch jitter)
    samples = {1: [], loop_reps: []}
    for it in range(reps + 1):
        for r in (1, loop_reps) if it % 2 == 0 else (loop_reps, 1):
            t0 = _time.perf_counter()
            for _ in range(inner):
                runners[r]()
            samples[r].append((_time.perf_counter() - t0) / inner)
    # drop the first sample pair (warm-up drift), pair the rest
    diffs = sorted(
        (b - a) / (loop_reps - 1) * 1e9
        for a, b in zip(samples[1][1:], samples[loop_reps][1:])
    )
    exec_ns = diffs[len(diffs) // 2]
    times = {1: min(samples[1]), loop_reps: min(samples[loop_reps]),
             "diffs_us": [round(d / 1000, 1) for d in diffs]}
    return exec_ns, times


def bench_exec_ns_chain(inputs, iters=32, reps=7):
    """Measure per-execution device time by emitting `iters` sequential
    bass_exec custom calls inside ONE jitted program, data-chained by
    feeding each execution's y output back as the next x input (same
    shape/dtype). The device runs the kernels back-to-back in a single
    dispatch, so exec = (T_chain - T_single) / (iters - 1) cancels the
    per-dispatch axon overhead and its (large) jitter."""
    import time as _time
    import jax
    from jax.experimental.shard_map import shard_map
    from jax.sharding import Mesh, PartitionSpec, NamedSharding
    from concourse import bass2jax

    x = np.asarray(inputs["x"], np.float32)
    c, arrs = _prep(inputs)
    key = tuple(sorted(c.items()))
    if key not in _CACHE:
        _CACHE[key] = _build(c)
    nc = _CACHE[key]
    bass2jax.install_neuronx_cc_hook()

    base = {
        "tqT": _to_bf16(arrs["tqT"]),
        "toT": _to_bf16(arrs["toT"]),
        "w1u": arrs["w1u"].astype(np.float32),
    }
    if c["need_g1"]:
        base["g1v"] = arrs["g1"]
    if c["need_b1"]:
        base["b1v"] = arrs["b1"]
    if c["need_bt"]:
        base["btv"] = arrs["bt"].astype(np.float32)
    in_maps = [dict(base, x=np.ascontiguousarray(_to_bf16(x[i]))) for i in range(B)]

    partition_name = nc.partition_id_tensor.name if nc.partition_id_tensor else None
    in_names, out_names, out_avals, zero_outs = [], [], [], []
    for alloc in nc.m.functions[0].allocations:
        if not isinstance(alloc, mybir.MemoryLocationSet):
            continue
        name = alloc.memorylocations[0].name
        if alloc.kind == "ExternalInput":
            if name != partition_name:
                in_names.append(name)
        elif alloc.kind == "ExternalOutput":
            out_names.append(name)
            shape = tuple(alloc.tensor_shape)
            dtype = mybir.dt.np(alloc.dtype)
            out_avals.append(jax.core.ShapedArray(shape, dtype))
            zero_outs.append(np.zeros(shape, dtype))
    n_params = len(in_names)
    bind_names = list(in_names) + list(out_names)
    if partition_name is not None:
        bind_names.append(partition_name)
    xi = in_names.index("x")
    yi = out_names.index("y")

    def _make_body(k):
        def _body(*args):
            operands = list(args)
            pid = [bass2jax.partition_id_tensor()] if partition_name else []
            outs = None
            for _ in range(k):
                outs = bass2jax._bass_exec_p.bind(
                    *(operands + pid),
                    out_avals=tuple(out_avals),
                    in_names=tuple(bind_names),
                    out_names=tuple(out_names),
                    lowering_input_output_aliases=(),
                    sim_require_finite=True,
                    sim_require_nnan=True,
                    nc=nc,
                )
                operands = list(operands)
                operands[xi] = outs[yi]  # serialize: next x <- this y
            return tuple(outs)
        return _body

    devices = jax.devices()[:B]
    mesh = Mesh(np.asarray(devices), ("core",))
    spec = PartitionSpec("core")
    n_out = len(out_names)
    per_core = [[np.asarray(m[nm]) for nm in in_names] for m in in_maps]
    concat_in = [
        np.concatenate([per_core[cc][i] for cc in range(B)], axis=0)
        for i in range(n_params)
    ]
    concat_zeros = [
        np.zeros((B * z.shape[0], *z.shape[1:]), z.dtype) for z in zero_outs
    ]
    dev_args = [
        jax.device_put(a, NamedSharding(mesh, spec)) for a in concat_in + concat_zeros
    ]

    fs = {}
    for k in (1, iters):
        fs[k] = jax.jit(
            shard_map(
                _make_body(k), mesh=mesh,
                in_specs=(spec,) * (n_params + n_out),
                out_specs=(spec,) * n_out,
                check_rep=False,
            )
        )
        jax.block_until_ready(fs[k](*dev_args))  # compile + warm

    # alternate k=1 / k=iters samples so slow drift cancels in the pairing
    samples = {1: [], iters: []}
    for _ in range(reps):
        for k in (1, iters):
            t0 = _time.perf_counter()
            jax.block_until_ready(fs[k](*dev_args))
            samples[k].append(_time.perf_counter() - t0)
    diffs = sorted(
        (b - a) / (iters - 1) * 1e9
        for a, b in zip(samples[1], samples[iters])
    )
    exec_ns = diffs[len(diffs) // 2]  # median paired difference
    times = {1: min(samples[1]), iters: min(samples[iters]),
             "diffs_us": [round(d / 1000, 1) for d in diffs]}
    return exec_ns, times


def bench_exec_ns(inputs, iters=32, reps=5, body_reps=1):
    """Measure per-execution NEFF time by chaining `iters` sequential
    executions inside one jitted program (chained through the output
    buffers) and comparing against a 1-execution program."""
    import time as _time
    import jax
    from jax.experimental.shard_map import shard_map
    from jax.sharding import Mesh, PartitionSpec, NamedSharding
    from concourse import bass2jax, mybir as _mybir

    x = np.asarray(inputs["x"], np.float32)
    c, arrs = _prep(inputs)
    if body_reps != 1:
        c["body_reps"] = body_reps
    key = tuple(sorted(c.items()))
    if key not in _CACHE:
        _CACHE[key] = _build(c)
    nc = _CACHE[key]
    bass2jax.install_neuronx_cc_hook()

    base = {
        "tqT": _to_bf16(arrs["tqT"]),
        "toT": _to_bf16(arrs["toT"]),
        "w1u": arrs["w1u"].astype(np.float32),
    }
    if c["need_g1"]:
        base["g1v"] = arrs["g1"]
    if c["need_b1"]:
        base["b1v"] = arrs["b1"]
    if c["need_bt"]:
        base["btv"] = arrs["bt"].astype(np.float32)
    in_maps = [dict(base, x=np.ascontiguousarray(_to_bf16(x[i]))) for i in range(B)]

    partition_name = nc.partition_id_tensor.name if nc.partition_id_tensor else None
    in_names, out_names, out_avals, zero_outs = [], [], [], []
    for alloc in nc.m.functions[0].allocations:
        if not isinstance(alloc, mybir.MemoryLocationSet):
            continue
        name = alloc.memorylocations[0].name
        if alloc.kind == "ExternalInput":
            if name != partition_name:
                in_names.append(name)
        elif alloc.kind == "ExternalOutput":
            out_names.append(name)
            shape = tuple(alloc.tensor_shape)
            dtype = mybir.dt.np(alloc.dtype)
            out_avals.append(jax.core.ShapedArray(shape, dtype))
            zero_outs.append(np.zeros(shape, dtype))
    n_params = len(in_names)

    bind_names = list(in_names) + list(out_names)
    if partition_name is not None:
        bind_names.append(partition_name)

    def _body(*args):
        operands = list(args)
        pid = [bass2jax.partition_id_tensor()] if partition_name else []
        outs = bass2jax._bass_exec_p.bind(
            *(operands + pid),
            out_avals=tuple(out_avals),
            in_names=tuple(bind_names),
            out_names=tuple(out_names),
            lowering_input_output_aliases=(),
            sim_require_finite=True,
            sim_require_nnan=True,
            nc=nc,
        )
        return tuple(outs)

    devices = jax.devices()[:B]
    mesh = Mesh(np.asarray(devices), ("core",))
    spec = PartitionSpec("core")
    n_out = len(out_names)
    per_core = [[np.asarray(m[nm]) for nm in in_names] for m in in_maps]
    concat_in = [
        np.concatenate([per_core[cc][i] for cc in range(B)], axis=0)
        for i in range(n_params)
    ]
    concat_zeros = [
        np.zeros((B * z.shape[0], *z.shape[1:]), z.dtype) for z in zero_outs
    ]
    dev_args = [
        jax.device_put(a, NamedSharding(mesh, spec)) for a in concat_in + concat_zeros
    ]

    f = jax.jit(
        shard_map(
            _body, mesh=mesh,
            in_specs=(spec,) * (n_params + n_out),
            out_specs=(spec,) * n_out,
            check_rep=False,
        )
    )
    jax.block_until_ready(f(*dev_args))  # compile + warm

    times = {}
    for k in (1, iters):
        best = float("inf")
        for _ in range(reps):
            t0 = _time.perf_counter()
            r = None
            for _ in range(k):
                r = f(*dev_args)  # async dispatch; device executes in-order
            jax.block_until_ready(r)
            best = min(best, _time.perf_counter() - t0)
        times[k] = best
    exec_ns = (times[iters] - times[1]) / (iters - 1) * 1e9
    return exec_ns, times



# revision 48
# speedup vs baseline: 1.1763x; 1.1042x over previous
"""Trainium2 Bass kernel for nn_Attention_6794638262338.

Single-layer attention block with BitNet-style ternary-quantized projections:
    x -> LN1 -> qkv proj (ternary W) -> MHA softmax -> LN2 -> out proj (ternary W)

Strategy: pure data parallelism. batch=8, n_cores=8 -> one batch element per
core, no collectives. Each core runs an identical Bass/Tile program.

Math folds (host side):
  - ternary_quant(W) = T * s with T in {-1,0,1}: pass T in bf16 (exact), fold
    s_qkv^2 * DIM_HEAD^-0.5 into the exp() activation scale, fold s_qkv/s_out
    into the LN2 rsqrt epsilon/scale.
  - softmax denominator: out = (sum_m exp(s)*v) / colsum. colsum obtained free
    by appending a ones-column to v in the attn@v matmul (M=65); division done
    via DVE reciprocal + GpSimd partition_broadcast + DVE multiply.
  - LN2: mean/var via ones-matmul column sums of a^T, tiny PE transposes to get
    per-row stats, y = (z - mu*W1) * rsqrt-ish using host-precomputed
    W1 = rowsum of effective output weight.
"""

import numpy as np
from contextlib import ExitStack

import concourse.bass as bass
import concourse.mybir as mybir
import concourse.tile as tile
from concourse import bacc
from concourse.bass import ts, ds
from concourse.bass_utils import run_bass_kernel_spmd
from concourse.masks import make_identity

F32 = mybir.dt.float32
BF16 = mybir.dt.bfloat16
FP8 = mybir.dt.float8e4
DR = mybir.MatmulPerfMode.DoubleRow
AF = mybir.ActivationFunctionType
ALU = mybir.AluOpType

B, N, D = 8, 1024, 512
H, DH = 8, 64
INNER = H * DH  # 512
NT = N // 128   # 8 n-tiles
DC = D // 128   # 4 d-chunks
EPS_LN = 1e-5
EPS_Q = 1e-6

TRACE = False          # set by test.py to capture an NTFF profile
LAST_RESULTS = None    # BassKernelResults of the most recent run

_CACHE = {}


def _ternary(w):
    """Replicate reference ternary_quant in fp32; return (unit ternary, scale)."""
    w = np.asarray(w, np.float32)
    s = np.float32(np.mean(np.abs(w), dtype=np.float32))
    t = np.round(np.clip(w / (s + np.float32(EPS_Q)), -1.0, 1.0)).astype(np.float32)
    return t, float(s)


def _emit(ctx: ExitStack, tc: "tile.TileContext", io: dict, c: dict, sfx: str = ""):
    nc = tc.nc
    dbg = c.get("debug", False)
    loop_reps = c.get("loop_reps", 0)

    def dump(name, ap):
        if dbg:
            d = nc.dram_tensor(f"dbg_{name}{sfx}", list(ap.shape), ap.dtype, kind="ExternalOutput").ap()
            nc.sync.dma_start(out=d, in_=ap)
    x, tqT, toT, w1u, y = io["x"], io["tqT"], io["toT"], io["w1u"], io["y"]

    need_g1 = c["need_g1"]
    need_b1 = c["need_b1"]
    need_bt = c["need_bt"]

    # ---------------- pools ----------------
    const_p = ctx.enter_context(tc.tile_pool(name="const" + sfx, bufs=1))
    xp = ctx.enter_context(tc.tile_pool(name="xp" + sfx, bufs=3))
    lnp = ctx.enter_context(tc.tile_pool(name="lnp" + sfx, bufs=4))
    xlnp = ctx.enter_context(tc.tile_pool(name="xlnp" + sfx, bufs=3))
    big = ctx.enter_context(tc.tile_pool(name="big" + sfx, bufs=1))
    attp = ctx.enter_context(tc.tile_pool(name="attp" + sfx, bufs=2))
    smp = ctx.enter_context(tc.tile_pool(name="smp" + sfx, bufs=3))
    outp = ctx.enter_context(tc.tile_pool(name="outp" + sfx, bufs=2))
    # PSUM budget: 8 banks = ps_s ([128,1024] x2 = 4) + ps_o ([65,512] x2 = 2)
    #              + ps_m ([128,512] x2 = 2)
    ps_s = ctx.enter_context(tc.tile_pool(name="ps_s" + sfx, bufs=2, space="PSUM"))
    ps_o = ctx.enter_context(tc.tile_pool(name="ps_o" + sfx, bufs=2, space="PSUM"))
    ps_m = ctx.enter_context(tc.tile_pool(name="ps_m" + sfx, bufs=2, space="PSUM"))

    # ---------------- constants ----------------
    ident = const_p.tile([128, 128], BF16)
    make_identity(nc, ident)
    # stats column: 1/INNER folded in, so the s1/s2 ones-matmuls produce
    # mu and E[a^2] directly (drops two ACT hops per LN2 group)
    ones128 = const_p.tile([128, 1], BF16)
    nc.vector.memset(ones128, 1.0 / INNER)
    eps1 = const_p.tile([128, 1], F32)
    nc.vector.memset(eps1, float(EPS_LN))
    eps2 = const_p.tile([128, 1], F32)
    nc.vector.memset(eps2, c["eps_eff"])
    # warm the ln/exp activation table while the first x tile is in flight
    warm = const_p.tile([128, 1], F32)
    nc.scalar.activation(warm, eps1, AF.Ln, bias=eps1)
    nc.scalar.activation(warm, warm, AF.Exp, scale=-0.5)

    # weight loads go on the GpSimd DMA queue so the x tiles own the SP
    # queue from t=0 (they gate the LN1->transpose critical path)
    # qkv unit-ternary weights, transposed: [d, 3*inner] -> sbuf [128, DC, 3*inner]
    tq_sb = const_p.tile([128, DC, 3 * INNER], BF16)
    nc.gpsimd.dma_start(out=tq_sb, in_=tqT.rearrange("(c p) o -> p c o", p=128))
    # out-proj unit weights (g2 folded), transposed: [o, dout] -> [128, DC, dout]
    toT_sb = const_p.tile([128, DC, INNER], BF16)
    nc.gpsimd.dma_start(out=toT_sb, in_=toT.rearrange("(c p) o -> p c o", p=128))
    # W1 rowsums broadcast across partitions
    w1b = const_p.tile([128, INNER], F32)
    nc.gpsimd.dma_start(
        out=w1b,
        in_=bass.AP(tensor=w1u.tensor, offset=w1u.offset, ap=[[0, 128]] + list(w1u.ap)),
    )
    if need_g1:
        g1_ap = io["g1v"]
        g1b = const_p.tile([128, D], F32)
        nc.gpsimd.dma_start(
            out=g1b,
            in_=bass.AP(tensor=g1_ap.tensor, offset=g1_ap.offset, ap=[[0, 128]] + list(g1_ap.ap)),
        )
    if need_b1:
        b1_ap = io["b1v"]
        b1b = const_p.tile([128, D], F32)
        nc.gpsimd.dma_start(
            out=b1b,
            in_=bass.AP(tensor=b1_ap.tensor, offset=b1_ap.offset, ap=[[0, 128]] + list(b1_ap.ap)),
        )
    if need_bt:
        bt_ap = io["btv"]
        btb = const_p.tile([128, INNER], F32)
        nc.gpsimd.dma_start(
            out=btb,
            in_=bass.AP(tensor=bt_ap.tensor, offset=bt_ap.offset, ap=[[0, 128]] + list(bt_ap.ap)),
        )

    scale_exp = c["scale_exp"]

    def body():
        # ---------------- persistent big tensors ----------------
        # xln^T: [d, n] bf16, split in two n-half tiles [128, DC, 512] so the
        # first qkv matmuls start after only half of Phase A
        xlnTh = [
            big.tile([128, DC, 512], BF16, name=f"xlnTh{i}", tag=f"xlnTh{i}")
            for i in range(2)
        ]
        # q^T, k^T head-major: [o, n] as [128, DC, N] (o = otile*128 + p)
        qT = big.tile([128, DC, N], BF16)
        kT = big.tile([128, DC, N], BF16)
        # v row-major with ones column: [128, mt, h, 65] (m = mt*128 + p)
        v_sb = big.tile([128, NT, H, DH + 1], BF16)
        nc.vector.memset(v_sb[:, :, :, DH : DH + 1], 1.0)
        # pair-stacked divided attention out: partition 0:64 = head 2p,
        # 64:128 = head 2p+1 (DVE cross-partition writes)
        aT2 = big.tile([128, DC, N], BF16)
        # squares of aT2 for the LN2 sum-of-squares (filled by GpSimd)
        sq_sb = big.tile([128, DC, N], BF16)

        def emit_qk(ot, nns=(0, 1)):
            # q, k head-major: psum[o_tile, n] = sum_dc Tq[:,dc,ot].T @ xlnT[:,dc,n]
            # k before q per n-half: the first scores matmul needs (kT nn0,
            # qT nn0) only. Copies run on ACT (Copy activation) to keep DVE
            # free for the fast-exp tiles + divides.
            for nn in nns:
                for sec, dst in ((1, kT), (0, qT)):
                    pq = ps_m.tile([128, 512], F32, name="pq", tag="mm")
                    for dc in range(DC):
                        nc.tensor.matmul(
                            pq,
                            lhsT=tq_sb[:, dc, ds(sec * INNER + ot * 128, 128)],
                            rhs=xlnTh[nn][:, dc, :],
                            start=(dc == 0), stop=(dc == DC - 1),
                        )
                    if c.get("qk_act", True):
                        nc.scalar.activation(out=dst[:, ot, ts(nn, 512)], in_=pq, func=AF.Copy)
                    else:
                        nc.vector.tensor_copy(out=dst[:, ot, ts(nn, 512)], in_=pq)


        # ================ Phase A: load x, LN1, transpose ================
        for nt in range(NT):
            # x arrives bf16 (host-converted): halves the input DMA bytes
            xt = xp.tile([128, D], BF16, name="xt", tag="xt")
            nc.sync.dma_start(out=xt, in_=x[ts(nt, 128), :])
            st6 = lnp.tile([128, 6], F32, name="st6", tag="st6")
            nc.vector.bn_stats(st6, xt)
            mv = lnp.tile([128, 2], F32, name="mv", tag="mv")
            nc.vector.bn_aggr(mv, st6)
            # rstd = exp(-0.5*ln(var+eps)) — keeps ACT on the ln/exp table set
            # (same set the attention exp uses; avoids sqrt-set thrashing)
            sd = lnp.tile([128, 1], F32, name="sd", tag="sd")
            nc.scalar.activation(sd, mv[:, 1:2], AF.Ln, bias=eps1)
            rs = lnp.tile([128, 1], F32, name="rs", tag="rs")
            nc.scalar.activation(rs, sd, AF.Exp, scale=-0.5)
            xl = xlnp.tile([128, D], BF16, name="xl", tag="xl")
            if need_g1 or need_b1:
                xlf = xlnp.tile([128, D], F32, name="xlf", tag="xlf")
                nc.vector.tensor_scalar(
                    out=xlf, in0=xt, scalar1=mv[:, 0:1], scalar2=rs,
                    op0=ALU.subtract, op1=ALU.mult,
                )
                if need_g1:
                    nc.vector.tensor_mul(xlf, xlf, g1b)
                if need_b1:
                    nc.vector.tensor_add(xlf, xlf, b1b)
                nc.vector.tensor_copy(xl, xlf)
            else:
                # (x - mu)*rs on ACT (idle in Phase A; DVE is the pacer):
                # Identity activation with per-partition scale rs and bias
                # -mu*rs
                nrsmu = lnp.tile([128, 1], F32, name="nrsmu", tag="nrsmu")
                nc.vector.tensor_scalar(
                    out=nrsmu, in0=mv[:, 0:1], scalar1=rs, scalar2=-1.0,
                    op0=ALU.mult, op1=ALU.mult,
                )
                nc.scalar.activation(out=xl, in_=xt, func=AF.Identity, scale=rs, bias=nrsmu)
            # transpose via matmul with identity: out = xl_slice.T. All four
            # d-chunks land in one psum tile -> one strided copy into xlnT.
            # Copies for the second half go to GpSimd: DVE is the Phase A
            # pacer and the weight DMAs have drained off the Pool queue by
            # then.
            pt = ps_m.tile([128, DC, 128], F32, name="pt", tag="mm")
            for dc in range(DC):
                nc.tensor.matmul(
                    pt[:, dc, :], lhsT=xl[:, ts(dc, 128)], rhs=ident, start=True, stop=True
                )
            if c.get("xlncp_act", False):
                nc.scalar.activation(out=xlnTh[nt // 4][:, :, ts(nt % 4, 128)], in_=pt, func=AF.Copy)
            else:
                nc.vector.tensor_copy(out=xlnTh[nt // 4][:, :, ts(nt % 4, 128)], in_=pt)
            if nt == 3:
                emit_qk(0, nns=(0,))  # first n-half of q/k as soon as it exists

        # ================ Phase B+C interleaved: qkv otiles feed attention
        # head-pairs as soon as their q/k tile is ready, so ACT starts exp()
        # early and stays the pacer without idle lead-in. ================
        def emit_v(mts):
            # v row-major: psum[m_tile, o] = sum_dc xlnT[:,dc,mt].T @ Tq_v[:,dc,:]
            for mt in mts:
                pv = ps_m.tile([128, 512], F32, name="pv", tag="mm")
                for dc in range(DC):
                    nc.tensor.matmul(
                        pv,
                        lhsT=xlnTh[mt // 4][:, dc, ts(mt % 4, 128)],
                        rhs=tq_sb[:, dc, ds(2 * INNER, INNER)],
                        start=(dc == 0), stop=(dc == DC - 1),
                    )
                # strided copy into per-head layout [128, h, 64]
                if c.get("vcp_act", False):
                    nc.scalar.activation(
                        out=v_sb[:, mt, :, 0:DH],
                        in_=pv.rearrange("p (h d) -> p h d", h=H), func=AF.Copy,
                    )
                else:
                    nc.vector.tensor_copy(
                        out=v_sb[:, mt, :, 0:DH],
                        in_=pv.rearrange("p (h d) -> p h d", h=H),
                    )

        # ~1/3 of the exp tiles run on DVE as a Schraudolph bf16-bit-trick
        # (bits = round(s*scale*log2e*128 + 16256-shift) as int16, bitcast to
        # bf16 ~= exp(s*scale) with ~3% sawtooth err that mostly cancels in
        # softmax). This splits the exp wall (the mid-kernel pacer) between
        # ACT and DVE. Tail-critical (mt=7) tiles stay on ACT.
        fexp_a, fexp_b = c["fexp_a"], c["fexp_b"]
        I16 = mybir.dt.int16

        _fexp_sets = {
            0: {0: (), 1: ()},
            16: {0: (1, 4), 1: (2, 5)},
            24: {0: (1, 3, 5), 1: (2, 4, 6)},
            32: {0: (1, 3, 5, 6), 1: (0, 2, 4, 5)},
            40: {0: (0, 1, 3, 5, 6), 1: (0, 2, 3, 4, 5)},
        }

        def fexp_on_dve(p, sub, mt):
            if not c.get("fexp", True):
                return False
            return mt in _fexp_sets[c.get("fexp_n", 24)][sub]

        def emit_scores_pair(p, interleave=None):
            """Scores+exp for heads 2p (partitions 0:64) and 2p+1 (64:128).
            atn is one tile per (sub, mt) so downstream attn@v matmuls only
            wait on the exps they actually read. `interleave(j)` is called
            after exp(mt=j+3) to slot tail-pair attn@v matmuls between
            score matmuls."""
            atns = {0: {}, 1: {}}
            for mt in range(NT):
                pss = [
                    ps_s.tile([128, N], F32, name="pssa", tag="s"),
                    ps_s.tile([128, N], F32, name="pssb", tag="s"),
                ]
                # sub-major order: each sub's two n-half matmuls are
                # adjacent, so its exp fires one matmul earlier
                for sub in range(2):
                    for nn in range(2):
                        base = sub * 64
                        nc.tensor.matmul(
                            pss[sub][:, ts(nn, 512)],
                            lhsT=kT[ds(base, 64), p, ts(mt, 128)],
                            rhs=qT[ds(base, 64), p, ts(nn, 512)],
                            start=True, stop=True,
                        )
                for sub in range(2):
                    if fexp_on_dve(p, sub, mt):
                        ai = attp.tile([128, N], I16, name=f"atn{sub}_{mt}", tag=f"atn{sub}_{mt}")
                        nc.vector.tensor_scalar(
                            out=ai, in0=pss[sub], scalar1=fexp_a, scalar2=fexp_b,
                            op0=ALU.mult, op1=ALU.add,
                        )
                        a = ai.bitcast(BF16)
                    else:
                        a = attp.tile([128, N], BF16, name=f"atn{sub}_{mt}", tag=f"atn{sub}_{mt}")
                        nc.scalar.activation(out=a, in_=pss[sub], func=AF.Exp, scale=scale_exp)
                    atns[sub][mt] = a
                if interleave is not None and mt >= 3:
                    interleave(mt - 3, atns)
            return atns

        def divide(h, po2):
            # divide chain straight off PSUM: reciprocal of the colsum row
            # (DVE reads psum p64, writes a partition-0 staging row — engines
            # CAN shift partitions, HW-verified) -> GpSimd partition_broadcast
            # (source must be in partitions 0..15: Q7 core 0 does the read)
            # -> multiply straight into aT2 rows 0:64 / 64:128 (cross-
            # partition DVE write kills the old odd-head remap DMA).
            rc0 = smp.tile([1, 2, 512], F32, name="rc0", tag="rc0")
            for nn in range(2):
                nc.vector.reciprocal(rc0[:, nn, :], po2[nn][64:65, :])
            rbt = smp.tile([64, 2, 512], F32, name="rbt", tag="rbt")
            if c.get("pbcast", False):
                nc.gpsimd.partition_broadcast(rbt, rc0, channels=64)
            else:
                bc_dram = nc.dram_tensor(f"cs_scratch{h}" + sfx, [2, 512], F32).ap()
                nc.sync.dma_start(out=bc_dram, in_=rc0)
                nc.sync.dma_start(
                    out=rbt,
                    in_=bass.AP(tensor=bc_dram.tensor, offset=bc_dram.offset,
                                ap=[[0, 64]] + list(bc_dram.ap)),
                )
            for nn in range(2):
                nc.vector.tensor_tensor(
                    out=aT2[ds(64 * (h % 2), 64), h // 2, ts(nn, 512)],
                    in0=po2[nn][0:64, :], in1=rbt[:, nn, :], op=ALU.mult,
                )

        def emit_out(h, atn, pool=None, tag="po"):
            po2 = [
                (pool or ps_o).tile([65, 512], F32, name=f"po{nn}", tag=tag)
                for nn in range(2)
            ]
            for mt in range(NT):
                for nn in range(2):
                    nc.tensor.matmul(
                        po2[nn],
                        lhsT=v_sb[:, mt, h, :],
                        rhs=atn[mt][:, ts(nn, 512)],
                        start=(mt == 0), stop=(mt == NT - 1),
                    )
            divide(h, po2)

        # driver: scores-pair 0 starts as soon as its q/k tile exists (ACT
        # starts exp'ing early); v and the next pair's q/k are emitted behind
        # the current pair's scores so PE fills its exp-wait slack with them;
        # out-matmuls run one pair behind. Squares for the LN2 sum-of-squares
        # run on idle GpSimd as chunks finish (last chunk on DVE: tail-critical).
        emit_qk(0, nns=(1,))  # nn0 was emitted inside Phase A at nt==3
        prev = emit_scores_pair(0)
        emit_v(range(0, 8))
        emit_qk(1)
        for pair in range(1, 3):
            atns = emit_scores_pair(pair)
            emit_qk(pair + 1)
            pp = pair - 1
            emit_out(2 * pp, prev[0])
            emit_out(2 * pp + 1, prev[1])
            nc.gpsimd.tensor_mul(sq_sb[:, pp, :], aT2[:, pp, :], aT2[:, pp, :])
            prev = atns

        # pair 3: head 6's attn@v accumulation is interleaved into the
        # scores loop three exp-steps behind (borrowing the ps_m slots,
        # idle until phase D), so only its last three accumulation steps
        # trail the final exp. Pair 2's out matmuls + divides drain from a
        # work queue a few per step so the PE load stays level and ACT
        # never starves. Head 7 runs after the loop as usual.
        po6 = [ps_m.tile([65, 512], F32, name=f"po6_{nn}", tag="mm") for nn in range(2)]

        po45 = {}
        pending = []

        def _alloc45(h):
            po45[h] = [
                ps_o.tile([65, 512], F32, name=f"po{nn}", tag="po") for nn in range(2)
            ]

        def _mm45(h, atn, mt):
            for nn in range(2):
                nc.tensor.matmul(
                    po45[h][nn],
                    lhsT=v_sb[:, mt, h, :],
                    rhs=atn[mt][:, ts(nn, 512)],
                    start=(mt == 0), stop=(mt == NT - 1),
                )

        for _h, _sub in ((4, 0), (5, 1)):
            pending.append((lambda h=_h: _alloc45(h)))
            for _mt in range(NT):
                pending.append(lambda h=_h, s=_sub, mt=_mt: _mm45(h, prev[s], mt))
            pending.append(lambda h=_h: divide(h, po45[h]))
        pending.append(lambda: nc.gpsimd.tensor_mul(sq_sb[:, 2, :], aT2[:, 2, :], aT2[:, 2, :]))

        def tail_out(j, atns3):
            for nn in range(2):
                nc.tensor.matmul(
                    po6[nn], lhsT=v_sb[:, j, 6, :], rhs=atns3[0][j][:, ts(nn, 512)],
                    start=(j == 0), stop=(j == NT - 1),
                )
            for _ in range(5):
                if pending:
                    pending.pop(0)()

        atns3 = emit_scores_pair(3, interleave=tail_out)
        while pending:
            pending.pop(0)()
        for j in range(5, 8):
            tail_out(j, atns3)
        # head 7 attn@v on ps_s slots (free after the last exps) so it does
        # not wait for head 5's divide to release a ps_o slot
        po7 = [ps_s.tile([65, 512], F32, name=f"po7_{nn}", tag="s") for nn in range(2)]
        for mt in range(NT):
            for nn in range(2):
                nc.tensor.matmul(
                    po7[nn], lhsT=v_sb[:, mt, 7, :], rhs=atns3[1][mt][:, ts(nn, 512)],
                    start=(mt == 0), stop=(mt == NT - 1),
                )
        divide(6, po6)
        divide(7, po7)
        nc.vector.tensor_mul(sq_sb[:, 3, :], aT2[:, 3, :], aT2[:, 3, :])

        dump("qT", qT)
        dump("kT", kT)
        dump("v", v_sb)
        dump("aT2", aT2)

        # ================ Phase D: LN2 stats + output projection ================
        # Four groups of 2 n-tiles. j=0's z goes to a [128, 513] ps_s tile
        # whose 513th column (toT_sb col 512 = 1/INNER) IS the s1 mean —
        # the s1 stats matmuls ride the projection for free. j=1 stays
        # [128, 512] on ps_m (1 bank) + explicit s1 matmuls, preserving the
        # 4-deep pz pipeline. The y ops avoid ACT entirely (Pool + DVE),
        # keeping ACT free for the exp wall.
        for g in range(4):
            # st[:, 0, j] = s1 (sum_o a), st[:, 1, j] = s2 (sum_o a^2)
            st = ps_o.tile([128, 2, 2], F32, name=f"st{g}", tag="po")
            pzs = []
            zsts = []
            for j in range(2):
                nt = 2 * g + j
                pz = (
                    ps_s.tile([128, INNER], F32, name="pz", tag="s")
                    if j == 0
                    else ps_m.tile([128, INNER], F32, name="pz", tag="mm")
                )
                pzs.append(pz)
                for ch in range(DC):
                    nc.tensor.matmul(
                        pz, lhsT=aT2[:, ch, ts(nt, 128)], rhs=toT_sb[:, ch, :],
                        start=(ch == 0), stop=(ch == DC - 1),
                    )
                    nc.tensor.matmul(
                        st[:, 0, j : j + 1], lhsT=aT2[:, ch, ts(nt, 128)], rhs=ones128,
                        start=(ch == 0), stop=(ch == DC - 1),
                    )
                for ch in range(DC):
                    nc.tensor.matmul(
                        st[:, 1, j : j + 1], lhsT=sq_sb[:, ch, ts(nt, 128)], rhs=ones128,
                        start=(ch == 0), stop=(ch == DC - 1),
                    )
                if j == 1:
                    # DVE-path tile stages z to SBUF; the ACT-path tile (j=0)
                    # reads its z PSUM directly via Identity
                    zst = outp.tile([128, INNER], BF16, name="zst", tag="zst", bufs=2)
                    nc.vector.tensor_copy(zst, pz)
                    zsts.append(zst)

            # the 1/INNER fold makes mu and E[a^2] direct; var = E[a^2]-mu^2;
            # r2 = s_o / sqrt(var + eps_eff). mu lifted to SBUF right away so
            # psum slots free early.
            muc = lnp.tile([128, 2], F32, name=f"muc{g}", tag="muc", bufs=2)
            nc.vector.tensor_copy(muc, st[:, 0, :])
            musq = lnp.tile([128, 2], F32, name=f"musq{g}", tag="musq", bufs=2)
            nc.vector.tensor_mul(musq, muc, muc)
            var = lnp.tile([128, 2], F32, name=f"var{g}", tag="var", bufs=2)
            nc.vector.tensor_sub(var, st[:, 1, :], musq)
            sd2 = lnp.tile([128, 2], F32, name=f"sd2{g}", tag="sd2", bufs=2)
            nc.scalar.activation(sd2, var, AF.Ln, bias=eps2, scale=c["inv_so2"])
            r2 = lnp.tile([128, 2], F32, name=f"r2{g}", tag="r2", bufs=2)
            nc.scalar.activation(r2, sd2, AF.Exp, scale=-0.5)
            r2n = lnp.tile([128, 2], F32, name=f"r2n{g}", tag="r2n", bufs=2)
            nc.vector.tensor_scalar_mul(r2n, r2, -1.0)
            # nmur2 = -mu*r2 for the rank-1 W1 term
            nmur2 = lnp.tile([128, 2], F32, name=f"nmur2{g}", tag="nmur2", bufs=2)
            nc.vector.tensor_mul(nmur2, muc, r2n)

            # y = (z - mu*W1) * r2 (+ bias_total)
            # j=0 (ACT path): y = Identity(z*r2) + Identity(W1*(-mu*r2)),
            #   summed on GpSimd — keeps the tail off DVE.
            # j=1 (DVE path): u = (W1*mu) - z ; y = u*(-r2)
            for j in range(2):
                nt = 2 * g + j
                # y is written bf16 (halves the output DMA bytes); the last
                # op of each path writes the bf16 tile directly
                yt = outp.tile([128, INNER], BF16, name="yt", tag="yt")
                if j == 0:
                    t1 = outp.tile([128, INNER], F32, name="t1", tag="t1", bufs=2)
                    nc.scalar.activation(out=t1, in_=pzs[0], func=AF.Identity,
                                         scale=r2[:, 0:1])
                    t2 = outp.tile([128, INNER], F32, name="t2", tag="t2", bufs=2)
                    nc.scalar.activation(out=t2, in_=w1b, func=AF.Identity,
                                         scale=nmur2[:, 0:1])
                    if need_bt:
                        ytf = outp.tile([128, INNER], F32, name="ytf", tag="ytf")
                        nc.gpsimd.tensor_add(ytf, t1, t2)
                        nc.gpsimd.tensor_add(yt, ytf, btb)
                    else:
                        nc.gpsimd.tensor_add(yt, t1, t2)
                    # j=0 y DMA rides the Pool queue (cheap issue, parallel
                    # with the j=1 DMA on the SP queue)
                    nc.gpsimd.dma_start(out=y[ts(nt, 128), :], in_=yt)
                    continue
                else:
                    ut = outp.tile([128, INNER], F32, name="ut", tag="ut")
                    nc.vector.scalar_tensor_tensor(
                        out=ut, in0=w1b, scalar=muc[:, 1:2], in1=zsts[0],
                        op0=ALU.mult, op1=ALU.subtract,
                    )
                    if need_bt:
                        nc.vector.tensor_scalar_mul(ut, ut, r2n[:, 1:2])
                        nc.vector.tensor_add(yt, ut, btb)
                    else:
                        nc.vector.tensor_scalar_mul(yt, ut, r2n[:, 1:2])
                nc.sync.dma_start(out=y[ts(nt, 128), :], in_=yt)

    if loop_reps:
        with tc.For_i(0, loop_reps):
            body()
    else:
        body()


def _build(c: dict):
    nc = bacc.Bacc("TRN2", target_bir_lowering=False, debug=False, num_devices=B)
    io = {
        "x": nc.dram_tensor("x", [N, D], BF16, kind="ExternalInput").ap(),
        "tqT": nc.dram_tensor("tqT", [D, 3 * INNER], BF16, kind="ExternalInput").ap(),
        "toT": nc.dram_tensor("toT", [INNER, INNER], BF16, kind="ExternalInput").ap(),
        "w1u": nc.dram_tensor("w1u", [INNER], F32, kind="ExternalInput").ap(),
        "y": nc.dram_tensor("y", [N, D], BF16, kind="ExternalOutput").ap(),
    }
    if c["need_g1"]:
        io["g1v"] = nc.dram_tensor("g1v", [D], F32, kind="ExternalInput").ap()
    if c["need_b1"]:
        io["b1v"] = nc.dram_tensor("b1v", [D], F32, kind="ExternalInput").ap()
    if c["need_bt"]:
        io["btv"] = nc.dram_tensor("btv", [INNER], F32, kind="ExternalInput").ap()
    reps = c.get("body_reps", 1)
    with tile.TileContext(nc) as tc:
        for r in range(reps):
            with ExitStack() as ctx:
                _emit(ctx, tc, io, c, sfx="" if r == 0 else f"_r{r}")

    nc.compile()

    # The act-table-load pass greedily picks the first set containing each
    # function, thrashing between `natural_log` (Ln) and `exp_and_others`
    # (Exp) on every rstd computation (18 reloads @ ~1.3-2.7us each). All
    # activation funcs this kernel uses (Ln, Exp, Copy, Identity) live
    # together in `natural_log_exp_and_others`, so rewrite the first load to
    # that set and drop the rest.
    from concourse.hw_specs import get_activation_tables
    tset = list(get_activation_tables(nc.m.arch).keys())
    nle = tset.index("natural_log_exp_and_others")
    for blk in nc.main_func.blocks:
        keep, first = [], False
        for inst in blk.instructions:
            if type(inst).__name__ == "InstLoadActFuncSet":
                si = getattr(inst, "sync_info", None)
                clean = si is None or (not si.on_wait and not si.on_update)
                if not first:
                    inst.act_func_set_id = nle
                    first = True
                    keep.append(inst)
                elif not clean:
                    inst.act_func_set_id = nle
                    keep.append(inst)
            else:
                keep.append(inst)
        blk.instructions[:] = keep
    return nc


def _prep(inputs):
    g1 = np.asarray(inputs["g1"], np.float32)
    b1 = np.asarray(inputs["b1"], np.float32)
    g2 = np.asarray(inputs["g2"], np.float32)
    b2 = np.asarray(inputs["b2"], np.float32)
    b_out = np.asarray(inputs["b_out"], np.float32)

    Tq, s_q = _ternary(inputs["W_qkv"])   # [3*inner, d]
    To, s_o = _ternary(inputs["W_out"])   # [dout, o]

    Wp = To * g2[None, :]                 # fold g2 (exact when g2 == 1)
    toT = np.ascontiguousarray(Wp.T)      # [o, dout]
    w1u = Wp.sum(axis=1).astype(np.float32)
    bias_total = (b2 @ To.T) * np.float32(s_o) + b_out

    LOG2E = 1.4426950408889634
    scale_exp = float(s_q * s_q * (DH ** -0.5))
    c = {
        "scale_exp": scale_exp,
        "fexp_a": float(scale_exp * LOG2E * 128.0),
        "fexp_b": float(16256.0 - 4.0),
        "inv_so2": float(1.0 / (s_o * s_o)),
        "eps_eff": float(EPS_LN / (s_q * s_q * s_o * s_o)),
        "need_g1": bool(not np.allclose(g1, 1.0)),
        "need_b1": bool(np.any(b1)),
        "need_bt": bool(np.any(bias_total)),
    }
    arrs = {
        "tqT": np.ascontiguousarray(Tq.T),
        "toT": toT,
        "w1u": w1u,
        "g1": g1, "b1": b1, "bt": bias_total,
    }
    return c, arrs


def _to_bf16(a):
    import ml_dtypes
    return np.asarray(a, np.float32).astype(ml_dtypes.bfloat16)


def _to_fp8(a):
    import ml_dtypes
    return np.asarray(a, np.float32).astype(ml_dtypes.float8_e4m3)


def kernel(**inputs) -> np.ndarray:
    global LAST_RESULTS
    x = np.asarray(inputs["x"], np.float32)
    assert x.shape == (B, N, D)
    c, arrs = _prep(inputs)

    key = tuple(sorted(c.items()))
    if key not in _CACHE:
        _CACHE[key] = _build(c)
    nc = _CACHE[key]

    base = {
        "tqT": _to_bf16(arrs["tqT"]),
        "toT": _to_bf16(arrs["toT"]),
        "w1u": arrs["w1u"].astype(np.float32),
    }
    if c["need_g1"]:
        base["g1v"] = arrs["g1"]
    if c["need_b1"]:
        base["b1v"] = arrs["b1"]
    if c["need_bt"]:
        base["btv"] = arrs["bt"].astype(np.float32)

    in_maps = [dict(base, x=np.ascontiguousarray(_to_bf16(x[i]))) for i in range(B)]
    res = run_bass_kernel_spmd(nc, in_maps, core_ids=list(range(B)), trace=TRACE)
    LAST_RESULTS = res
    out = np.stack([res.results[i]["y"] for i in range(B)], axis=0)
    return out.astype(np.float32)


def _pjrt_runner(nc, in_maps):
    """Build a jitted single-execution runner for a compiled Bass module on
    the 8 axon cores. Returns a 0-arg callable that runs + blocks."""
    import jax
    from jax.experimental.shard_map import shard_map
    from jax.sharding import Mesh, PartitionSpec, NamedSharding
    from concourse import bass2jax

    bass2jax.install_neuronx_cc_hook()
    partition_name = nc.partition_id_tensor.name if nc.partition_id_tensor else None
    in_names, out_names, out_avals, zero_outs = [], [], [], []
    for alloc in nc.m.functions[0].allocations:
        if not isinstance(alloc, mybir.MemoryLocationSet):
            continue
        name = alloc.memorylocations[0].name
        if alloc.kind == "ExternalInput":
            if name != partition_name:
                in_names.append(name)
        elif alloc.kind == "ExternalOutput":
            out_names.append(name)
            shape = tuple(alloc.tensor_shape)
            dtype = mybir.dt.np(alloc.dtype)
            out_avals.append(jax.core.ShapedArray(shape, dtype))
            zero_outs.append(np.zeros(shape, dtype))
    n_params = len(in_names)
    bind_names = list(in_names) + list(out_names)
    if partition_name is not None:
        bind_names.append(partition_name)

    def _body(*args):
        operands = list(args)
        pid = [bass2jax.partition_id_tensor()] if partition_name else []
        outs = bass2jax._bass_exec_p.bind(
            *(operands + pid),
            out_avals=tuple(out_avals),
            in_names=tuple(bind_names),
            out_names=tuple(out_names),
            lowering_input_output_aliases=(),
            sim_require_finite=True,
            sim_require_nnan=True,
            nc=nc,
        )
        return tuple(outs)

    devices = jax.devices()[:B]
    mesh = Mesh(np.asarray(devices), ("core",))
    spec = PartitionSpec("core")
    n_out = len(out_names)
    per_core = [[np.asarray(m[nm]) for nm in in_names] for m in in_maps]
    concat_in = [
        np.concatenate([per_core[cc][i] for cc in range(B)], axis=0)
        for i in range(n_params)
    ]
    concat_zeros = [
        np.zeros((B * z.shape[0], *z.shape[1:]), z.dtype) for z in zero_outs
    ]
    dev_args = [
        jax.device_put(a, NamedSharding(mesh, spec)) for a in concat_in + concat_zeros
    ]
    f = jax.jit(
        shard_map(
            _body, mesh=mesh,
            in_specs=(spec,) * (n_params + n_out),
            out_specs=(spec,) * n_out,
            check_rep=False,
        )
    )

    def run():
        jax.block_until_ready(f(*dev_args))

    run()  # compile + warm
    return run


def _bench_in_maps(inputs):
    x = np.asarray(inputs["x"], np.float32)
    c, arrs = _prep(inputs)
    base = {
        "tqT": _to_bf16(arrs["tqT"]),
        "toT": _to_bf16(arrs["toT"]),
        "w1u": arrs["w1u"].astype(np.float32),
    }
    if c["need_g1"]:
        base["g1v"] = arrs["g1"]
    if c["need_b1"]:
        base["b1v"] = arrs["b1"]
    if c["need_bt"]:
        base["btv"] = arrs["bt"].astype(np.float32)
    return c, [dict(base, x=np.ascontiguousarray(_to_bf16(x[i]))) for i in range(B)]


def bench_exec_ns_loop(inputs, loop_reps=129, reps=9):
    """Measure device exec time with a hardware For_i loop around the kernel
    body: one dispatch runs the body `loop_reps` times back-to-back on
    device, so exec = (T_loop - T_single) / (loop_reps - 1) with dispatch
    overhead cancelled and amortized over a large R."""
    import time as _time

    c, in_maps = _bench_in_maps(inputs)
    runners = {}
    for r in (1, loop_reps):
        cr = dict(c, loop_reps=r)
        key = tuple(sorted(cr.items()))
        if key not in _CACHE:
            _CACHE[key] = _build(cr)
        runners[r] = _pjrt_runner(_CACHE[key], in_maps)

    inner = 2  # calls per timing sample (averages dispatch jitter)
    samples = {1: [], loop_reps: []}
    for it in range(reps + 1):
        for r in (1, loop_reps) if it % 2 == 0 else (loop_reps, 1):
            t0 = _time.perf_counter()
            for _ in range(inner):
                runners[r]()
            samples[r].append((_time.perf_counter() - t0) / inner)
    # drop the first sample pair (warm-up drift), pair the rest
    diffs = sorted(
        (b - a) / (loop_reps - 1) * 1e9
        for a, b in zip(samples[1][1:], samples[loop_reps][1:])
    )
    exec_ns = diffs[len(diffs) // 2]
    times = {1: min(samples[1]), loop_reps: min(samples[loop_reps]),
             "diffs_us": [round(d / 1000, 1) for d in diffs]}
    return exec_ns, times


def bench_exec_ns_chain(inputs, iters=32, reps=7):
    """Measure per-execution device time by emitting `iters` sequential
    bass_exec custom calls inside ONE jitted program, data-chained by
    feeding each execution's y output back as the next x input (same
    shape/dtype). The device runs the kernels back-to-back in a single
    dispatch, so exec = (T_chain - T_single) / (iters - 1) cancels the
    per-dispatch axon overhead and its (large) jitter."""
    import time as _time
    import jax
    from jax.experimental.shard_map import shard_map
    from jax.sharding import Mesh, PartitionSpec, NamedSharding
    from concourse import bass2jax

    x = np.asarray(inputs["x"], np.float32)
    c, arrs = _prep(inputs)
    key = tuple(sorted(c.items()))
    if key not in _CACHE:
        _CACHE[key] = _build(c)
    nc = _CACHE[key]
    bass2jax.install_neuronx_cc_hook()

    base = {
        "tqT": _to_bf16(arrs["tqT"]),
        "toT": _to_bf16(arrs["toT"]),
        "w1u": arrs["w1u"].astype(np.float32),
    }
    if c["need_g1"]:
        base["g1v"] = arrs["g1"]
    if c["need_b1"]:
        base["b1v"] = arrs["b1"]
    if c["need_bt"]:
        base["btv"] = arrs["bt"].astype(np.float32)
    in_maps = [dict(base, x=np.ascontiguousarray(_to_bf16(x[i]))) for i in range(B)]

    partition_name = nc.partition_id_tensor.name if nc.partition_id_tensor else None
    in_names, out_names, out_avals, zero_outs = [], [], [], []
    for alloc in nc.m.functions[0].allocations:
        if not isinstance(alloc, mybir.MemoryLocationSet):
            continue
        name = alloc.memorylocations[0].name
        if alloc.kind == "ExternalInput":
            if name != partition_name:
                in_names.append(name)
        elif alloc.kind == "ExternalOutput":
            out_names.append(name)
            shape = tuple(alloc.tensor_shape)
            dtype = mybir.dt.np(alloc.dtype)
            out_avals.append(jax.core.ShapedArray(shape, dtype))
            zero_outs.append(np.zeros(shape, dtype))
    n_params = len(in_names)
    bind_names = list(in_names) + list(out_names)
    if partition_name is not None:
        bind_names.append(partition_name)
    xi = in_names.index("x")
    yi = out_names.index("y")

    def _make_body(k):
        def _body(*args):
            operands = list(args)
            pid = [bass2jax.partition_id_tensor()] if partition_name else []
            outs = None
            for _ in range(k):
                outs = bass2jax._bass_exec_p.bind(
                    *(operands + pid),
                    out_avals=tuple(out_avals),
                    in_names=tuple(bind_names),
                    out_names=tuple(out_names),
                    lowering_input_output_aliases=(),
                    sim_require_finite=True,
                    sim_require_nnan=True,
                    nc=nc,
                )
                operands = list(operands)
                operands[xi] = outs[yi]  # serialize: next x <- this y
            return tuple(outs)
        return _body

    devices = jax.devices()[:B]
    mesh = Mesh(np.asarray(devices), ("core",))
    spec = PartitionSpec("core")
    n_out = len(out_names)
    per_core = [[np.asarray(m[nm]) for nm in in_names] for m in in_maps]
    concat_in = [
        np.concatenate([per_core[cc][i] for cc in range(B)], axis=0)
        for i in range(n_params)
    ]
    concat_zeros = [
        np.zeros((B * z.shape[0], *z.shape[1:]), z.dtype) for z in zero_outs
    ]
    dev_args = [
        jax.device_put(a, NamedSharding(mesh, spec)) for a in concat_in + concat_zeros
    ]

    fs = {}
    for k in (1, iters):
        fs[k] = jax.jit(
            shard_map(
                _make_body(k), mesh=mesh,
                in_specs=(spec,) * (n_params + n_out),
                out_specs=(spec,) * n_out,
                check_rep=False,
            )
        )
        jax.block_until_ready(fs[k](*dev_args))  # compile + warm

    # alternate k=1 / k=iters samples so slow drift cancels in the pairing
    samples = {1: [], iters: []}
    for _ in range(reps):
        for k in (1, iters):
            t0 = _time.perf_counter()
            jax.block_until_ready(fs[k](*dev_args))
            samples[k].append(_time.perf_counter() - t0)
    diffs = sorted(
        (b - a) / (iters - 1) * 1e9
        for a, b in zip(samples[1], samples[iters])
    )
    exec_ns = diffs[len(diffs) // 2]  # median paired difference
    times = {1: min(samples[1]), iters: min(samples[iters]),
             "diffs_us": [round(d / 1000, 1) for d in diffs]}
    return exec_ns, times


def bench_exec_ns(inputs, iters=32, reps=5, body_reps=1):
    """Measure per-execution NEFF time by chaining `iters` sequential
    executions inside one jitted program (chained through the output
    buffers) and comparing against a 1-execution program."""
    import time as _time
    import jax
    from jax.experimental.shard_map import shard_map
    from jax.sharding import Mesh, PartitionSpec, NamedSharding
    from concourse import bass2jax, mybir as _mybir

    x = np.asarray(inputs["x"], np.float32)
    c, arrs = _prep(inputs)
    if body_reps != 1:
        c["body_reps"] = body_reps
    key = tuple(sorted(c.items()))
    if key not in _CACHE:
        _CACHE[key] = _build(c)
    nc = _CACHE[key]
    bass2jax.install_neuronx_cc_hook()

    base = {
        "tqT": _to_bf16(arrs["tqT"]),
        "toT": _to_bf16(arrs["toT"]),
        "w1u": arrs["w1u"].astype(np.float32),
    }
    if c["need_g1"]:
        base["g1v"] = arrs["g1"]
    if c["need_b1"]:
        base["b1v"] = arrs["b1"]
    if c["need_bt"]:
        base["btv"] = arrs["bt"].astype(np.float32)
    in_maps = [dict(base, x=np.ascontiguousarray(_to_bf16(x[i]))) for i in range(B)]

    partition_name = nc.partition_id_tensor.name if nc.partition_id_tensor else None
    in_names, out_names, out_avals, zero_outs = [], [], [], []
    for alloc in nc.m.functions[0].allocations:
        if not isinstance(alloc, mybir.MemoryLocationSet):
            continue
        name = alloc.memorylocations[0].name
        if alloc.kind == "ExternalInput":
            if name != partition_name:
                in_names.append(name)
        elif alloc.kind == "ExternalOutput":
            out_names.append(name)
            shape = tuple(alloc.tensor_shape)
            dtype = mybir.dt.np(alloc.dtype)
            out_avals.append(jax.core.ShapedArray(shape, dtype))
            zero_outs.append(np.zeros(shape, dtype))
    n_params = len(in_names)

    bind_names = list(in_names) + list(out_names)
    if partition_name is not None:
        bind_names.append(partition_name)

    def _body(*args):
        operands = list(args)
        pid = [bass2jax.partition_id_tensor()] if partition_name else []
        outs = bass2jax._bass_exec_p.bind(
            *(operands + pid),
            out_avals=tuple(out_avals),
            in_names=tuple(bind_names),
            out_names=tuple(out_names),
            lowering_input_output_aliases=(),
            sim_require_finite=True,
            sim_require_nnan=True,
            nc=nc,
        )
        return tuple(outs)

    devices = jax.devices()[:B]
    mesh = Mesh(np.asarray(devices), ("core",))
    spec = PartitionSpec("core")
    n_out = len(out_names)
    per_core = [[np.asarray(m[nm]) for nm in in_names] for m in in_maps]
    concat_in = [
        np.concatenate([per_core[cc][i] for cc in range(B)], axis=0)
        for i in range(n_params)
    ]
    concat_zeros = [
        np.zeros((B * z.shape[0], *z.shape[1:]), z.dtype) for z in zero_outs
    ]
    dev_args = [
        jax.device_put(a, NamedSharding(mesh, spec)) for a in concat_in + concat_zeros
    ]

    f = jax.jit(
        shard_map(
            _body, mesh=mesh,
            in_specs=(spec,) * (n_params + n_out),
            out_specs=(spec,) * n_out,
            check_rep=False,
        )
    )
    jax.block_until_ready(f(*dev_args))  # compile + warm

    times = {}
    for k in (1, iters):
        best = float("inf")
        for _ in range(reps):
            t0 = _time.perf_counter()
            r = None
            for _ in range(k):
                r = f(*dev_args)  # async dispatch; device executes in-order
            jax.block_until_ready(r)
            best = min(best, _time.perf_counter() - t0)
        times[k] = best
    exec_ns = (times[iters] - times[1]) / (iters - 1) * 1e9
    return exec_ns, times



# revision 49
# speedup vs baseline: 1.2010x; 1.0210x over previous
"""Trainium2 Bass kernel for nn_Attention_6794638262338.

Single-layer attention block with BitNet-style ternary-quantized projections:
    x -> LN1 -> qkv proj (ternary W) -> MHA softmax -> LN2 -> out proj (ternary W)

Strategy: pure data parallelism. batch=8, n_cores=8 -> one batch element per
core, no collectives. Each core runs an identical Bass/Tile program.

Math folds (host side):
  - ternary_quant(W) = T * s with T in {-1,0,1}: pass T in bf16 (exact), fold
    s_qkv^2 * DIM_HEAD^-0.5 into the exp() activation scale, fold s_qkv/s_out
    into the LN2 rsqrt epsilon/scale.
  - softmax denominator: out = (sum_m exp(s)*v) / colsum. colsum obtained free
    by appending a ones-column to v in the attn@v matmul (M=65); division:
    DVE reciprocal (cross-partition write to a partition-0 row) -> DRAM-bounce
    broadcast DMA -> DVE multiply straight into aT2 (cross-partition writes,
    no remap DMA).
  - exp split: ~24/64 softmax tiles run on DVE as a Schraudolph bf16 bit-trick
    (int16 tensor_scalar + bitcast), relieving the ACT exp wall; q/k psum->sbuf
    copies ride ACT (Copy) to backfill. x/y are bf16 at the DRAM boundary.
  - LN2: mean/var via ones-matmul column sums of a^T, y = (z - mu*W1) * rsqrt
    using host-precomputed W1 = rowsum of effective output weight.

HW-tuning notes (validated by interleaved loop benches, see memory):
  fp8 DoubleRow (hi/lo or single) loses on ldweights cost / accuracy;
  gpsimd partition_broadcast is slower than the DMA bounce; Pool tensor_scalar
  and PSUM-reading stt in the y-path regress; loop_reps>129 hits throttling.
"""

import numpy as np
from contextlib import ExitStack

import concourse.bass as bass
import concourse.mybir as mybir
import concourse.tile as tile
from concourse import bacc
from concourse.bass import ts, ds
from concourse.bass_utils import run_bass_kernel_spmd
from concourse.masks import make_identity

F32 = mybir.dt.float32
BF16 = mybir.dt.bfloat16
AF = mybir.ActivationFunctionType
ALU = mybir.AluOpType

B, N, D = 8, 1024, 512
H, DH = 8, 64
INNER = H * DH  # 512
NT = N // 128   # 8 n-tiles
DC = D // 128   # 4 d-chunks
EPS_LN = 1e-5
EPS_Q = 1e-6

TRACE = False          # set by test.py to capture an NTFF profile
LAST_RESULTS = None    # BassKernelResults of the most recent run

_CACHE = {}


def _ternary(w):
    """Replicate reference ternary_quant in fp32; return (unit ternary, scale)."""
    w = np.asarray(w, np.float32)
    s = np.float32(np.mean(np.abs(w), dtype=np.float32))
    t = np.round(np.clip(w / (s + np.float32(EPS_Q)), -1.0, 1.0)).astype(np.float32)
    return t, float(s)


def _emit(ctx: ExitStack, tc: "tile.TileContext", io: dict, c: dict, sfx: str = ""):
    nc = tc.nc
    dbg = c.get("debug", False)
    loop_reps = c.get("loop_reps", 0)

    def dump(name, ap):
        if dbg:
            d = nc.dram_tensor(f"dbg_{name}{sfx}", list(ap.shape), ap.dtype, kind="ExternalOutput").ap()
            nc.sync.dma_start(out=d, in_=ap)
    x, tqT, toT, w1u, y = io["x"], io["tqT"], io["toT"], io["w1u"], io["y"]

    need_g1 = c["need_g1"]
    need_b1 = c["need_b1"]
    need_bt = c["need_bt"]

    # ---------------- pools ----------------
    const_p = ctx.enter_context(tc.tile_pool(name="const" + sfx, bufs=1))
    xp = ctx.enter_context(tc.tile_pool(name="xp" + sfx, bufs=3))
    lnp = ctx.enter_context(tc.tile_pool(name="lnp" + sfx, bufs=4))
    xlnp = ctx.enter_context(tc.tile_pool(name="xlnp" + sfx, bufs=3))
    big = ctx.enter_context(tc.tile_pool(name="big" + sfx, bufs=1))
    attp = ctx.enter_context(tc.tile_pool(name="attp" + sfx, bufs=2))
    smp = ctx.enter_context(tc.tile_pool(name="smp" + sfx, bufs=3))
    outp = ctx.enter_context(tc.tile_pool(name="outp" + sfx, bufs=2))
    # PSUM budget: 8 banks = ps_s ([128,1024] x2 = 4) + ps_o ([65,512] x2 = 2)
    #              + ps_m ([128,512] x2 = 2)
    ps_s = ctx.enter_context(tc.tile_pool(name="ps_s" + sfx, bufs=2, space="PSUM"))
    ps_o = ctx.enter_context(tc.tile_pool(name="ps_o" + sfx, bufs=2, space="PSUM"))
    ps_m = ctx.enter_context(tc.tile_pool(name="ps_m" + sfx, bufs=2, space="PSUM"))

    # ---------------- constants ----------------
    ident = const_p.tile([128, 128], BF16)
    make_identity(nc, ident)
    # stats column: 1/INNER folded in, so the s1/s2 ones-matmuls produce
    # mu and E[a^2] directly (drops two ACT hops per LN2 group)
    ones128 = const_p.tile([128, 1], BF16)
    nc.vector.memset(ones128, 1.0 / INNER)
    eps1 = const_p.tile([128, 1], F32)
    nc.vector.memset(eps1, float(EPS_LN))
    eps2 = const_p.tile([128, 1], F32)
    nc.vector.memset(eps2, c["eps_eff"])
    # warm the ln/exp activation table while the first x tile is in flight
    warm = const_p.tile([128, 1], F32)
    nc.scalar.activation(warm, eps1, AF.Ln, bias=eps1)
    nc.scalar.activation(warm, warm, AF.Exp, scale=-0.5)

    # weight loads go on the GpSimd DMA queue so the x tiles own the SP
    # queue from t=0 (they gate the LN1->transpose critical path)
    # qkv unit-ternary weights, transposed: [d, 3*inner] -> sbuf [128, DC, 3*inner]
    tq_sb = const_p.tile([128, DC, 3 * INNER], BF16)
    nc.gpsimd.dma_start(out=tq_sb, in_=tqT.rearrange("(c p) o -> p c o", p=128))
    # out-proj unit weights (g2 folded), transposed: [o, dout] -> [128, DC, dout]
    toT_sb = const_p.tile([128, DC, INNER], BF16)
    nc.gpsimd.dma_start(out=toT_sb, in_=toT.rearrange("(c p) o -> p c o", p=128))
    # W1 rowsums broadcast across partitions
    w1b = const_p.tile([128, INNER], F32)
    nc.gpsimd.dma_start(
        out=w1b,
        in_=bass.AP(tensor=w1u.tensor, offset=w1u.offset, ap=[[0, 128]] + list(w1u.ap)),
    )
    if need_g1:
        g1_ap = io["g1v"]
        g1b = const_p.tile([128, D], F32)
        nc.gpsimd.dma_start(
            out=g1b,
            in_=bass.AP(tensor=g1_ap.tensor, offset=g1_ap.offset, ap=[[0, 128]] + list(g1_ap.ap)),
        )
    if need_b1:
        b1_ap = io["b1v"]
        b1b = const_p.tile([128, D], F32)
        nc.gpsimd.dma_start(
            out=b1b,
            in_=bass.AP(tensor=b1_ap.tensor, offset=b1_ap.offset, ap=[[0, 128]] + list(b1_ap.ap)),
        )
    if need_bt:
        bt_ap = io["btv"]
        btb = const_p.tile([128, INNER], F32)
        nc.gpsimd.dma_start(
            out=btb,
            in_=bass.AP(tensor=bt_ap.tensor, offset=bt_ap.offset, ap=[[0, 128]] + list(bt_ap.ap)),
        )

    scale_exp = c["scale_exp"]

    def body():
        # ---------------- persistent big tensors ----------------
        # xln^T: [d, n] bf16, split in two n-half tiles [128, DC, 512] so the
        # first qkv matmuls start after only half of Phase A
        xlnTh = [
            big.tile([128, DC, 512], BF16, name=f"xlnTh{i}", tag=f"xlnTh{i}")
            for i in range(2)
        ]
        # q^T, k^T head-major: [o, n] as [128, DC, N] (o = otile*128 + p)
        qT = big.tile([128, DC, N], BF16)
        kT = big.tile([128, DC, N], BF16)
        # v row-major with ones column: [128, mt, h, 65] (m = mt*128 + p)
        v_sb = big.tile([128, NT, H, DH + 1], BF16)
        nc.vector.memset(v_sb[:, :, :, DH : DH + 1], 1.0)
        # pair-stacked divided attention out: partition 0:64 = head 2p,
        # 64:128 = head 2p+1 (DVE cross-partition writes)
        aT2 = big.tile([128, DC, N], BF16)
        # squares of aT2 for the LN2 sum-of-squares (filled by GpSimd)
        sq_sb = big.tile([128, DC, N], BF16)

        def emit_qk(ot, nns=(0, 1)):
            # q, k head-major: psum[o_tile, n] = sum_dc Tq[:,dc,ot].T @ xlnT[:,dc,n]
            # k before q per n-half: the first scores matmul needs (kT nn0,
            # qT nn0) only. Copies run on ACT (Copy activation) to keep DVE
            # free for the fast-exp tiles + divides.
            for nn in nns:
                for sec, dst in ((1, kT), (0, qT)):
                    pq = ps_m.tile([128, 512], F32, name="pq", tag="mm")
                    for dc in range(DC):
                        nc.tensor.matmul(
                            pq,
                            lhsT=tq_sb[:, dc, ds(sec * INNER + ot * 128, 128)],
                            rhs=xlnTh[nn][:, dc, :],
                            start=(dc == 0), stop=(dc == DC - 1),
                        )
                    if c.get("qk_act", True):
                        nc.scalar.activation(out=dst[:, ot, ts(nn, 512)], in_=pq, func=AF.Copy)
                    else:
                        nc.vector.tensor_copy(out=dst[:, ot, ts(nn, 512)], in_=pq)


        # ================ Phase A: load x, LN1, transpose ================
        for nt in range(NT):
            # x arrives bf16 (host-converted): halves the input DMA bytes
            xt = xp.tile([128, D], BF16, name="xt", tag="xt")
            nc.sync.dma_start(out=xt, in_=x[ts(nt, 128), :])
            st6 = lnp.tile([128, 6], F32, name="st6", tag="st6")
            nc.vector.bn_stats(st6, xt)
            mv = lnp.tile([128, 2], F32, name="mv", tag="mv")
            nc.vector.bn_aggr(mv, st6)
            # rstd = exp(-0.5*ln(var+eps)) — keeps ACT on the ln/exp table set
            # (same set the attention exp uses; avoids sqrt-set thrashing)
            sd = lnp.tile([128, 1], F32, name="sd", tag="sd")
            nc.scalar.activation(sd, mv[:, 1:2], AF.Ln, bias=eps1)
            rs = lnp.tile([128, 1], F32, name="rs", tag="rs")
            nc.scalar.activation(rs, sd, AF.Exp, scale=-0.5)
            xl = xlnp.tile([128, D], BF16, name="xl", tag="xl")
            if need_g1 or need_b1:
                xlf = xlnp.tile([128, D], F32, name="xlf", tag="xlf")
                nc.vector.tensor_scalar(
                    out=xlf, in0=xt, scalar1=mv[:, 0:1], scalar2=rs,
                    op0=ALU.subtract, op1=ALU.mult,
                )
                if need_g1:
                    nc.vector.tensor_mul(xlf, xlf, g1b)
                if need_b1:
                    nc.vector.tensor_add(xlf, xlf, b1b)
                nc.vector.tensor_copy(xl, xlf)
            else:
                # (x - mu)*rs on ACT (idle in Phase A; DVE is the pacer):
                # Identity activation with per-partition scale rs and bias
                # -mu*rs
                nrsmu = lnp.tile([128, 1], F32, name="nrsmu", tag="nrsmu")
                nc.vector.tensor_scalar(
                    out=nrsmu, in0=mv[:, 0:1], scalar1=rs, scalar2=-1.0,
                    op0=ALU.mult, op1=ALU.mult,
                )
                nc.scalar.activation(out=xl, in_=xt, func=AF.Identity, scale=rs, bias=nrsmu)
            # transpose via matmul with identity: out = xl_slice.T. All four
            # d-chunks land in one psum tile -> one strided copy into xlnT.
            # Copies for the second half go to GpSimd: DVE is the Phase A
            # pacer and the weight DMAs have drained off the Pool queue by
            # then.
            pt = ps_m.tile([128, DC, 128], F32, name="pt", tag="mm")
            for dc in range(DC):
                nc.tensor.matmul(
                    pt[:, dc, :], lhsT=xl[:, ts(dc, 128)], rhs=ident, start=True, stop=True
                )
            if c.get("xlncp_act", False):
                nc.scalar.activation(out=xlnTh[nt // 4][:, :, ts(nt % 4, 128)], in_=pt, func=AF.Copy)
            else:
                nc.vector.tensor_copy(out=xlnTh[nt // 4][:, :, ts(nt % 4, 128)], in_=pt)
            if nt == 3:
                emit_qk(0, nns=(0,))  # first n-half of q/k as soon as it exists

        # ================ Phase B+C interleaved: qkv otiles feed attention
        # head-pairs as soon as their q/k tile is ready, so ACT starts exp()
        # early and stays the pacer without idle lead-in. ================
        def emit_v(mts):
            # v row-major: psum[m_tile, o] = sum_dc xlnT[:,dc,mt].T @ Tq_v[:,dc,:]
            for mt in mts:
                pv = ps_m.tile([128, 512], F32, name="pv", tag="mm")
                for dc in range(DC):
                    nc.tensor.matmul(
                        pv,
                        lhsT=xlnTh[mt // 4][:, dc, ts(mt % 4, 128)],
                        rhs=tq_sb[:, dc, ds(2 * INNER, INNER)],
                        start=(dc == 0), stop=(dc == DC - 1),
                    )
                # strided copy into per-head layout [128, h, 64]
                if c.get("vcp_act", False):
                    nc.scalar.activation(
                        out=v_sb[:, mt, :, 0:DH],
                        in_=pv.rearrange("p (h d) -> p h d", h=H), func=AF.Copy,
                    )
                else:
                    nc.vector.tensor_copy(
                        out=v_sb[:, mt, :, 0:DH],
                        in_=pv.rearrange("p (h d) -> p h d", h=H),
                    )

        # ~1/3 of the exp tiles run on DVE as a Schraudolph bf16-bit-trick
        # (bits = round(s*scale*log2e*128 + 16256-shift) as int16, bitcast to
        # bf16 ~= exp(s*scale) with ~3% sawtooth err that mostly cancels in
        # softmax). This splits the exp wall (the mid-kernel pacer) between
        # ACT and DVE. Tail-critical (mt=7) tiles stay on ACT.
        fexp_a, fexp_b = c["fexp_a"], c["fexp_b"]
        I16 = mybir.dt.int16

        _fexp_sets = {
            0: {0: (), 1: ()},
            16: {0: (1, 4), 1: (2, 5)},
            24: {0: (1, 3, 5), 1: (2, 4, 6)},
            32: {0: (1, 3, 5, 6), 1: (0, 2, 4, 5)},
            40: {0: (0, 1, 3, 5, 6), 1: (0, 2, 3, 4, 5)},
        }

        def fexp_on_dve(p, sub, mt):
            if not c.get("fexp", True):
                return False
            return mt in _fexp_sets[c.get("fexp_n", 24)][sub]

        def emit_scores_pair(p, interleave=None):
            """Scores+exp for heads 2p (partitions 0:64) and 2p+1 (64:128).
            atn is one tile per (sub, mt) so downstream attn@v matmuls only
            wait on the exps they actually read. `interleave(j)` is called
            after exp(mt=j+3) to slot tail-pair attn@v matmuls between
            score matmuls."""
            atns = {0: {}, 1: {}}
            for mt in range(NT):
                pss = [
                    ps_s.tile([128, N], F32, name="pssa", tag="s"),
                    ps_s.tile([128, N], F32, name="pssb", tag="s"),
                ]
                # sub-major order: each sub's two n-half matmuls are
                # adjacent, so its exp fires one matmul earlier
                for sub in range(2):
                    for nn in range(2):
                        base = sub * 64
                        nc.tensor.matmul(
                            pss[sub][:, ts(nn, 512)],
                            lhsT=kT[ds(base, 64), p, ts(mt, 128)],
                            rhs=qT[ds(base, 64), p, ts(nn, 512)],
                            start=True, stop=True,
                        )
                for sub in range(2):
                    if fexp_on_dve(p, sub, mt):
                        ai = attp.tile([128, N], I16, name=f"atn{sub}_{mt}", tag=f"atn{sub}_{mt}")
                        nc.vector.tensor_scalar(
                            out=ai, in0=pss[sub], scalar1=fexp_a, scalar2=fexp_b,
                            op0=ALU.mult, op1=ALU.add,
                        )
                        a = ai.bitcast(BF16)
                    else:
                        a = attp.tile([128, N], BF16, name=f"atn{sub}_{mt}", tag=f"atn{sub}_{mt}")
                        nc.scalar.activation(out=a, in_=pss[sub], func=AF.Exp, scale=scale_exp)
                    atns[sub][mt] = a
                if interleave is not None and mt >= 3:
                    interleave(mt - 3, atns)
            return atns

        def divide(h, po2):
            # divide chain straight off PSUM: reciprocal of the colsum row
            # (DVE reads psum p64, writes a partition-0 staging row — engines
            # CAN shift partitions, HW-verified) -> GpSimd partition_broadcast
            # (source must be in partitions 0..15: Q7 core 0 does the read)
            # -> multiply straight into aT2 rows 0:64 / 64:128 (cross-
            # partition DVE write kills the old odd-head remap DMA).
            rc0 = smp.tile([1, 2, 512], F32, name="rc0", tag="rc0")
            for nn in range(2):
                nc.vector.reciprocal(rc0[:, nn, :], po2[nn][64:65, :])
            rbt = smp.tile([64, 2, 512], F32, name="rbt", tag="rbt")
            if c.get("pbcast", False):
                nc.gpsimd.partition_broadcast(rbt, rc0, channels=64)
            else:
                bc_dram = nc.dram_tensor(f"cs_scratch{h}" + sfx, [2, 512], F32).ap()
                nc.sync.dma_start(out=bc_dram, in_=rc0)
                nc.sync.dma_start(
                    out=rbt,
                    in_=bass.AP(tensor=bc_dram.tensor, offset=bc_dram.offset,
                                ap=[[0, 64]] + list(bc_dram.ap)),
                )
            for nn in range(2):
                nc.vector.tensor_tensor(
                    out=aT2[ds(64 * (h % 2), 64), h // 2, ts(nn, 512)],
                    in0=po2[nn][0:64, :], in1=rbt[:, nn, :], op=ALU.mult,
                )

        def emit_out(h, atn, pool=None, tag="po"):
            po2 = [
                (pool or ps_o).tile([65, 512], F32, name=f"po{nn}", tag=tag)
                for nn in range(2)
            ]
            for mt in range(NT):
                for nn in range(2):
                    nc.tensor.matmul(
                        po2[nn],
                        lhsT=v_sb[:, mt, h, :],
                        rhs=atn[mt][:, ts(nn, 512)],
                        start=(mt == 0), stop=(mt == NT - 1),
                    )
            divide(h, po2)

        # driver: scores-pair 0 starts as soon as its q/k tile exists (ACT
        # starts exp'ing early); v and the next pair's q/k are emitted behind
        # the current pair's scores so PE fills its exp-wait slack with them;
        # out-matmuls run one pair behind. Squares for the LN2 sum-of-squares
        # run on idle GpSimd as chunks finish (last chunk on DVE: tail-critical).
        emit_qk(0, nns=(1,))  # nn0 was emitted inside Phase A at nt==3
        prev = emit_scores_pair(0)
        emit_v(range(0, 8))
        emit_qk(1)
        for pair in range(1, 3):
            atns = emit_scores_pair(pair)
            emit_qk(pair + 1)
            pp = pair - 1
            emit_out(2 * pp, prev[0])
            emit_out(2 * pp + 1, prev[1])
            nc.gpsimd.tensor_mul(sq_sb[:, pp, :], aT2[:, pp, :], aT2[:, pp, :])
            prev = atns

        # pair 3: head 6's attn@v accumulation is interleaved into the
        # scores loop three exp-steps behind (borrowing the ps_m slots,
        # idle until phase D), so only its last three accumulation steps
        # trail the final exp. Pair 2's out matmuls + divides drain from a
        # work queue a few per step so the PE load stays level and ACT
        # never starves. Head 7 runs after the loop as usual.
        po6 = [ps_m.tile([65, 512], F32, name=f"po6_{nn}", tag="mm") for nn in range(2)]

        po45 = {}
        pending = []

        def _alloc45(h):
            po45[h] = [
                ps_o.tile([65, 512], F32, name=f"po{nn}", tag="po") for nn in range(2)
            ]

        def _mm45(h, atn, mt):
            for nn in range(2):
                nc.tensor.matmul(
                    po45[h][nn],
                    lhsT=v_sb[:, mt, h, :],
                    rhs=atn[mt][:, ts(nn, 512)],
                    start=(mt == 0), stop=(mt == NT - 1),
                )

        for _h, _sub in ((4, 0), (5, 1)):
            pending.append((lambda h=_h: _alloc45(h)))
            for _mt in range(NT):
                pending.append(lambda h=_h, s=_sub, mt=_mt: _mm45(h, prev[s], mt))
            pending.append(lambda h=_h: divide(h, po45[h]))
        pending.append(lambda: nc.gpsimd.tensor_mul(sq_sb[:, 2, :], aT2[:, 2, :], aT2[:, 2, :]))

        def tail_out(j, atns3):
            for nn in range(2):
                nc.tensor.matmul(
                    po6[nn], lhsT=v_sb[:, j, 6, :], rhs=atns3[0][j][:, ts(nn, 512)],
                    start=(j == 0), stop=(j == NT - 1),
                )
            for _ in range(5):
                if pending:
                    pending.pop(0)()

        atns3 = emit_scores_pair(3, interleave=tail_out)
        while pending:
            pending.pop(0)()
        for j in range(5, 8):
            tail_out(j, atns3)
        # head 7 attn@v on ps_s slots (free after the last exps) so it does
        # not wait for head 5's divide to release a ps_o slot
        po7 = [ps_s.tile([65, 512], F32, name=f"po7_{nn}", tag="s") for nn in range(2)]
        for mt in range(NT):
            for nn in range(2):
                nc.tensor.matmul(
                    po7[nn], lhsT=v_sb[:, mt, 7, :], rhs=atns3[1][mt][:, ts(nn, 512)],
                    start=(mt == 0), stop=(mt == NT - 1),
                )
        divide(6, po6)
        divide(7, po7)
        nc.vector.tensor_mul(sq_sb[:, 3, :], aT2[:, 3, :], aT2[:, 3, :])

        dump("qT", qT)
        dump("kT", kT)
        dump("v", v_sb)
        dump("aT2", aT2)

        # ================ Phase D: LN2 stats + output projection ================
        # Four groups of 2 n-tiles. j=0's z goes to a [128, 513] ps_s tile
        # whose 513th column (toT_sb col 512 = 1/INNER) IS the s1 mean —
        # the s1 stats matmuls ride the projection for free. j=1 stays
        # [128, 512] on ps_m (1 bank) + explicit s1 matmuls, preserving the
        # 4-deep pz pipeline. The y ops avoid ACT entirely (Pool + DVE),
        # keeping ACT free for the exp wall.
        for g in range(4):
            # st[:, 0, j] = s1 (sum_o a), st[:, 1, j] = s2 (sum_o a^2)
            st = ps_o.tile([128, 2, 2], F32, name=f"st{g}", tag="po")
            pzs = []
            zsts = []
            for j in range(2):
                nt = 2 * g + j
                pz = (
                    ps_s.tile([128, INNER], F32, name="pz", tag="s")
                    if j == 0
                    else ps_m.tile([128, INNER], F32, name="pz", tag="mm")
                )
                pzs.append(pz)
                for ch in range(DC):
                    nc.tensor.matmul(
                        pz, lhsT=aT2[:, ch, ts(nt, 128)], rhs=toT_sb[:, ch, :],
                        start=(ch == 0), stop=(ch == DC - 1),
                    )
                    nc.tensor.matmul(
                        st[:, 0, j : j + 1], lhsT=aT2[:, ch, ts(nt, 128)], rhs=ones128,
                        start=(ch == 0), stop=(ch == DC - 1),
                    )
                for ch in range(DC):
                    nc.tensor.matmul(
                        st[:, 1, j : j + 1], lhsT=sq_sb[:, ch, ts(nt, 128)], rhs=ones128,
                        start=(ch == 0), stop=(ch == DC - 1),
                    )
                if j == 1:
                    # DVE-path tile stages z to SBUF; the ACT-path tile (j=0)
                    # reads its z PSUM directly via Identity
                    zst = outp.tile([128, INNER], BF16, name="zst", tag="zst", bufs=2)
                    nc.vector.tensor_copy(zst, pz)
                    zsts.append(zst)

            # the 1/INNER fold makes mu and E[a^2] direct; var = E[a^2]-mu^2;
            # r2 = s_o / sqrt(var + eps_eff). mu lifted to SBUF right away so
            # psum slots free early.
            muc = lnp.tile([128, 2], F32, name=f"muc{g}", tag="muc", bufs=2)
            nc.vector.tensor_copy(muc, st[:, 0, :])
            musq = lnp.tile([128, 2], F32, name=f"musq{g}", tag="musq", bufs=2)
            nc.vector.tensor_mul(musq, muc, muc)
            var = lnp.tile([128, 2], F32, name=f"var{g}", tag="var", bufs=2)
            nc.vector.tensor_sub(var, st[:, 1, :], musq)
            sd2 = lnp.tile([128, 2], F32, name=f"sd2{g}", tag="sd2", bufs=2)
            nc.scalar.activation(sd2, var, AF.Ln, bias=eps2, scale=c["inv_so2"])
            r2 = lnp.tile([128, 2], F32, name=f"r2{g}", tag="r2", bufs=2)
            nc.scalar.activation(r2, sd2, AF.Exp, scale=-0.5)
            r2n = lnp.tile([128, 2], F32, name=f"r2n{g}", tag="r2n", bufs=2)
            nc.vector.tensor_scalar_mul(r2n, r2, -1.0)
            # nmur2 = -mu*r2 for the rank-1 W1 term
            nmur2 = lnp.tile([128, 2], F32, name=f"nmur2{g}", tag="nmur2", bufs=2)
            nc.vector.tensor_mul(nmur2, muc, r2n)

            # y = (z - mu*W1) * r2 (+ bias_total)
            # j=0 (ACT path): y = Identity(z*r2) + Identity(W1*(-mu*r2)),
            #   summed on GpSimd — keeps the tail off DVE.
            # j=1 (DVE path): u = (W1*mu) - z ; y = u*(-r2)
            for j in range(2):
                nt = 2 * g + j
                # y is written bf16 (halves the output DMA bytes); the last
                # op of each path writes the bf16 tile directly
                yt = outp.tile([128, INNER], BF16, name="yt", tag="yt")
                if j == 0:
                    t1 = outp.tile([128, INNER], F32, name="t1", tag="t1", bufs=2)
                    nc.scalar.activation(out=t1, in_=pzs[0], func=AF.Identity,
                                         scale=r2[:, 0:1])
                    t2 = outp.tile([128, INNER], F32, name="t2", tag="t2", bufs=2)
                    nc.scalar.activation(out=t2, in_=w1b, func=AF.Identity,
                                         scale=nmur2[:, 0:1])
                    if need_bt:
                        ytf = outp.tile([128, INNER], F32, name="ytf", tag="ytf")
                        nc.gpsimd.tensor_add(ytf, t1, t2)
                        nc.gpsimd.tensor_add(yt, ytf, btb)
                    else:
                        nc.gpsimd.tensor_add(yt, t1, t2)
                    # j=0 y DMA rides the Pool queue (cheap issue, parallel
                    # with the j=1 DMA on the SP queue)
                    nc.gpsimd.dma_start(out=y[ts(nt, 128), :], in_=yt)
                    continue
                else:
                    ut = outp.tile([128, INNER], F32, name="ut", tag="ut")
                    nc.vector.scalar_tensor_tensor(
                        out=ut, in0=w1b, scalar=muc[:, 1:2], in1=zsts[0],
                        op0=ALU.mult, op1=ALU.subtract,
                    )
                    if need_bt:
                        nc.vector.tensor_scalar_mul(ut, ut, r2n[:, 1:2])
                        nc.vector.tensor_add(yt, ut, btb)
                    else:
                        nc.vector.tensor_scalar_mul(yt, ut, r2n[:, 1:2])
                nc.sync.dma_start(out=y[ts(nt, 128), :], in_=yt)

    if loop_reps:
        with tc.For_i(0, loop_reps):
            body()
    else:
        body()


def _build(c: dict):
    nc = bacc.Bacc("TRN2", target_bir_lowering=False, debug=False, num_devices=B)
    io = {
        "x": nc.dram_tensor("x", [N, D], BF16, kind="ExternalInput").ap(),
        "tqT": nc.dram_tensor("tqT", [D, 3 * INNER], BF16, kind="ExternalInput").ap(),
        "toT": nc.dram_tensor("toT", [INNER, INNER], BF16, kind="ExternalInput").ap(),
        "w1u": nc.dram_tensor("w1u", [INNER], F32, kind="ExternalInput").ap(),
        "y": nc.dram_tensor("y", [N, D], BF16, kind="ExternalOutput").ap(),
    }
    if c["need_g1"]:
        io["g1v"] = nc.dram_tensor("g1v", [D], F32, kind="ExternalInput").ap()
    if c["need_b1"]:
        io["b1v"] = nc.dram_tensor("b1v", [D], F32, kind="ExternalInput").ap()
    if c["need_bt"]:
        io["btv"] = nc.dram_tensor("btv", [INNER], F32, kind="ExternalInput").ap()
    reps = c.get("body_reps", 1)
    with tile.TileContext(nc) as tc:
        for r in range(reps):
            with ExitStack() as ctx:
                _emit(ctx, tc, io, c, sfx="" if r == 0 else f"_r{r}")

    nc.compile()

    # The act-table-load pass greedily picks the first set containing each
    # function, thrashing between `natural_log` (Ln) and `exp_and_others`
    # (Exp) on every rstd computation (18 reloads @ ~1.3-2.7us each). All
    # activation funcs this kernel uses (Ln, Exp, Copy, Identity) live
    # together in `natural_log_exp_and_others`, so rewrite the first load to
    # that set and drop the rest.
    from concourse.hw_specs import get_activation_tables
    tset = list(get_activation_tables(nc.m.arch).keys())
    nle = tset.index("natural_log_exp_and_others")
    for blk in nc.main_func.blocks:
        keep, first = [], False
        for inst in blk.instructions:
            if type(inst).__name__ == "InstLoadActFuncSet":
                si = getattr(inst, "sync_info", None)
                clean = si is None or (not si.on_wait and not si.on_update)
                if not first:
                    inst.act_func_set_id = nle
                    first = True
                    keep.append(inst)
                elif not clean:
                    inst.act_func_set_id = nle
                    keep.append(inst)
            else:
                keep.append(inst)
        blk.instructions[:] = keep
    return nc


def _prep(inputs):
    g1 = np.asarray(inputs["g1"], np.float32)
    b1 = np.asarray(inputs["b1"], np.float32)
    g2 = np.asarray(inputs["g2"], np.float32)
    b2 = np.asarray(inputs["b2"], np.float32)
    b_out = np.asarray(inputs["b_out"], np.float32)

    Tq, s_q = _ternary(inputs["W_qkv"])   # [3*inner, d]
    To, s_o = _ternary(inputs["W_out"])   # [dout, o]

    Wp = To * g2[None, :]                 # fold g2 (exact when g2 == 1)
    toT = np.ascontiguousarray(Wp.T)      # [o, dout]
    w1u = Wp.sum(axis=1).astype(np.float32)
    bias_total = (b2 @ To.T) * np.float32(s_o) + b_out

    LOG2E = 1.4426950408889634
    scale_exp = float(s_q * s_q * (DH ** -0.5))
    c = {
        "scale_exp": scale_exp,
        "fexp_a": float(scale_exp * LOG2E * 128.0),
        "fexp_b": float(16256.0 - 4.0),
        "inv_so2": float(1.0 / (s_o * s_o)),
        "eps_eff": float(EPS_LN / (s_q * s_q * s_o * s_o)),
        "need_g1": bool(not np.allclose(g1, 1.0)),
        "need_b1": bool(np.any(b1)),
        "need_bt": bool(np.any(bias_total)),
    }
    arrs = {
        "tqT": np.ascontiguousarray(Tq.T),
        "toT": toT,
        "w1u": w1u,
        "g1": g1, "b1": b1, "bt": bias_total,
    }
    return c, arrs


def _to_bf16(a):
    import ml_dtypes
    return np.asarray(a, np.float32).astype(ml_dtypes.bfloat16)


def _to_fp8(a):
    import ml_dtypes
    return np.asarray(a, np.float32).astype(ml_dtypes.float8_e4m3)


def kernel(**inputs) -> np.ndarray:
    global LAST_RESULTS
    x = np.asarray(inputs["x"], np.float32)
    assert x.shape == (B, N, D)
    c, arrs = _prep(inputs)

    key = tuple(sorted(c.items()))
    if key not in _CACHE:
        _CACHE[key] = _build(c)
    nc = _CACHE[key]

    base = {
        "tqT": _to_bf16(arrs["tqT"]),
        "toT": _to_bf16(arrs["toT"]),
        "w1u": arrs["w1u"].astype(np.float32),
    }
    if c["need_g1"]:
        base["g1v"] = arrs["g1"]
    if c["need_b1"]:
        base["b1v"] = arrs["b1"]
    if c["need_bt"]:
        base["btv"] = arrs["bt"].astype(np.float32)

    in_maps = [dict(base, x=np.ascontiguousarray(_to_bf16(x[i]))) for i in range(B)]
    res = run_bass_kernel_spmd(nc, in_maps, core_ids=list(range(B)), trace=TRACE)
    LAST_RESULTS = res
    out = np.stack([res.results[i]["y"] for i in range(B)], axis=0)
    return out.astype(np.float32)


def _pjrt_runner(nc, in_maps):
    """Build a jitted single-execution runner for a compiled Bass module on
    the 8 axon cores. Returns a 0-arg callable that runs + blocks."""
    import jax
    from jax.experimental.shard_map import shard_map
    from jax.sharding import Mesh, PartitionSpec, NamedSharding
    from concourse import bass2jax

    bass2jax.install_neuronx_cc_hook()
    partition_name = nc.partition_id_tensor.name if nc.partition_id_tensor else None
    in_names, out_names, out_avals, zero_outs = [], [], [], []
    for alloc in nc.m.functions[0].allocations:
        if not isinstance(alloc, mybir.MemoryLocationSet):
            continue
        name = alloc.memorylocations[0].name
        if alloc.kind == "ExternalInput":
            if name != partition_name:
                in_names.append(name)
        elif alloc.kind == "ExternalOutput":
            out_names.append(name)
            shape = tuple(alloc.tensor_shape)
            dtype = mybir.dt.np(alloc.dtype)
            out_avals.append(jax.core.ShapedArray(shape, dtype))
            zero_outs.append(np.zeros(shape, dtype))
    n_params = len(in_names)
    bind_names = list(in_names) + list(out_names)
    if partition_name is not None:
        bind_names.append(partition_name)

    def _body(*args):
        operands = list(args)
        pid = [bass2jax.partition_id_tensor()] if partition_name else []
        outs = bass2jax._bass_exec_p.bind(
            *(operands + pid),
            out_avals=tuple(out_avals),
            in_names=tuple(bind_names),
            out_names=tuple(out_names),
            lowering_input_output_aliases=(),
            sim_require_finite=True,
            sim_require_nnan=True,
            nc=nc,
        )
        return tuple(outs)

    devices = jax.devices()[:B]
    mesh = Mesh(np.asarray(devices), ("core",))
    spec = PartitionSpec("core")
    n_out = len(out_names)
    per_core = [[np.asarray(m[nm]) for nm in in_names] for m in in_maps]
    concat_in = [
        np.concatenate([per_core[cc][i] for cc in range(B)], axis=0)
        for i in range(n_params)
    ]
    concat_zeros = [
        np.zeros((B * z.shape[0], *z.shape[1:]), z.dtype) for z in zero_outs
    ]
    dev_args = [
        jax.device_put(a, NamedSharding(mesh, spec)) for a in concat_in + concat_zeros
    ]
    f = jax.jit(
        shard_map(
            _body, mesh=mesh,
            in_specs=(spec,) * (n_params + n_out),
            out_specs=(spec,) * n_out,
            check_rep=False,
        )
    )

    def run():
        jax.block_until_ready(f(*dev_args))

    run()  # compile + warm
    return run


def _bench_in_maps(inputs):
    x = np.asarray(inputs["x"], np.float32)
    c, arrs = _prep(inputs)
    base = {
        "tqT": _to_bf16(arrs["tqT"]),
        "toT": _to_bf16(arrs["toT"]),
        "w1u": arrs["w1u"].astype(np.float32),
    }
    if c["need_g1"]:
        base["g1v"] = arrs["g1"]
    if c["need_b1"]:
        base["b1v"] = arrs["b1"]
    if c["need_bt"]:
        base["btv"] = arrs["bt"].astype(np.float32)
    return c, [dict(base, x=np.ascontiguousarray(_to_bf16(x[i]))) for i in range(B)]


def bench_exec_ns_loop(inputs, loop_reps=129, reps=9):
    """Measure device exec time with a hardware For_i loop around the kernel
    body: one dispatch runs the body `loop_reps` times back-to-back on
    device, so exec = (T_loop - T_single) / (loop_reps - 1) with dispatch
    overhead cancelled and amortized over a large R."""
    import time as _time

    c, in_maps = _bench_in_maps(inputs)
    runners = {}
    for r in (1, loop_reps):
        cr = dict(c, loop_reps=r)
        key = tuple(sorted(cr.items()))
        if key not in _CACHE:
            _CACHE[key] = _build(cr)
        runners[r] = _pjrt_runner(_CACHE[key], in_maps)

    inner = 2  # calls per timing sample (averages dispatch jitter)
    samples = {1: [], loop_reps: []}
    for it in range(reps + 1):
        for r in (1, loop_reps) if it % 2 == 0 else (loop_reps, 1):
            t0 = _time.perf_counter()
            for _ in range(inner):
                runners[r]()
            samples[r].append((_time.perf_counter() - t0) / inner)
    # drop the first sample pair (warm-up drift), pair the rest
    diffs = sorted(
        (b - a) / (loop_reps - 1) * 1e9
        for a, b in zip(samples[1][1:], samples[loop_reps][1:])
    )
    exec_ns = diffs[len(diffs) // 2]
    times = {1: min(samples[1]), loop_reps: min(samples[loop_reps]),
             "diffs_us": [round(d / 1000, 1) for d in diffs]}
    return exec_ns, times


def bench_exec_ns_chain(inputs, iters=32, reps=7):
    """Measure per-execution device time by emitting `iters` sequential
    bass_exec custom calls inside ONE jitted program, data-chained by
    feeding each execution's y output back as the next x input (same
    shape/dtype). The device runs the kernels back-to-back in a single
    dispatch, so exec = (T_chain - T_single) / (iters - 1) cancels the
    per-dispatch axon overhead and its (large) jitter."""
    import time as _time
    import jax
    from jax.experimental.shard_map import shard_map
    from jax.sharding import Mesh, PartitionSpec, NamedSharding
    from concourse import bass2jax

    x = np.asarray(inputs["x"], np.float32)
    c, arrs = _prep(inputs)
    key = tuple(sorted(c.items()))
    if key not in _CACHE:
        _CACHE[key] = _build(c)
    nc = _CACHE[key]
    bass2jax.install_neuronx_cc_hook()

    base = {
        "tqT": _to_bf16(arrs["tqT"]),
        "toT": _to_bf16(arrs["toT"]),
        "w1u": arrs["w1u"].astype(np.float32),
    }
    if c["need_g1"]:
        base["g1v"] = arrs["g1"]
    if c["need_b1"]:
        base["b1v"] = arrs["b1"]
    if c["need_bt"]:
        base["btv"] = arrs["bt"].astype(np.float32)
    in_maps = [dict(base, x=np.ascontiguousarray(_to_bf16(x[i]))) for i in range(B)]

    partition_name = nc.partition_id_tensor.name if nc.partition_id_tensor else None
    in_names, out_names, out_avals, zero_outs = [], [], [], []
    for alloc in nc.m.functions[0].allocations:
        if not isinstance(alloc, mybir.MemoryLocationSet):
            continue
        name = alloc.memorylocations[0].name
        if alloc.kind == "ExternalInput":
            if name != partition_name:
                in_names.append(name)
        elif alloc.kind == "ExternalOutput":
            out_names.append(name)
            shape = tuple(alloc.tensor_shape)
            dtype = mybir.dt.np(alloc.dtype)
            out_avals.append(jax.core.ShapedArray(shape, dtype))
            zero_outs.append(np.zeros(shape, dtype))
    n_params = len(in_names)
    bind_names = list(in_names) + list(out_names)
    if partition_name is not None:
        bind_names.append(partition_name)
    xi = in_names.index("x")
    yi = out_names.index("y")

    def _make_body(k):
        def _body(*args):
            operands = list(args)
            pid = [bass2jax.partition_id_tensor()] if partition_name else []
            outs = None
            for _ in range(k):
                outs = bass2jax._bass_exec_p.bind(
                    *(operands + pid),
                    out_avals=tuple(out_avals),
                    in_names=tuple(bind_names),
                    out_names=tuple(out_names),
                    lowering_input_output_aliases=(),
                    sim_require_finite=True,
                    sim_require_nnan=True,
                    nc=nc,
                )
                operands = list(operands)
                operands[xi] = outs[yi]  # serialize: next x <- this y
            return tuple(outs)
        return _body

    devices = jax.devices()[:B]
    mesh = Mesh(np.asarray(devices), ("core",))
    spec = PartitionSpec("core")
    n_out = len(out_names)
    per_core = [[np.asarray(m[nm]) for nm in in_names] for m in in_maps]
    concat_in = [
        np.concatenate([per_core[cc][i] for cc in range(B)], axis=0)
        for i in range(n_params)
    ]
    concat_zeros = [
        np.zeros((B * z.shape[0], *z.shape[1:]), z.dtype) for z in zero_outs
    ]
    dev_args = [
        jax.device_put(a, NamedSharding(mesh, spec)) for a in concat_in + concat_zeros
    ]

    fs = {}
    for k in (1, iters):
        fs[k] = jax.jit(
            shard_map(
                _make_body(k), mesh=mesh,
                in_specs=(spec,) * (n_params + n_out),
                out_specs=(spec,) * n_out,
                check_rep=False,
            )
        )
        jax.block_until_ready(fs[k](*dev_args))  # compile + warm

    # alternate k=1 / k=iters samples so slow drift cancels in the pairing
    samples = {1: [], iters: []}
    for _ in range(reps):
        for k in (1, iters):
            t0 = _time.perf_counter()
            jax.block_until_ready(fs[k](*dev_args))
            samples[k].append(_time.perf_counter() - t0)
    diffs = sorted(
        (b - a) / (iters - 1) * 1e9
        for a, b in zip(samples[1], samples[iters])
    )
    exec_ns = diffs[len(diffs) // 2]  # median paired difference
    times = {1: min(samples[1]), iters: min(samples[iters]),
             "diffs_us": [round(d / 1000, 1) for d in diffs]}
    return exec_ns, times


def bench_exec_ns(inputs, iters=32, reps=5, body_reps=1):
    """Measure per-execution NEFF time by chaining `iters` sequential
    executions inside one jitted program (chained through the output
    buffers) and comparing against a 1-execution program."""
    import time as _time
    import jax
    from jax.experimental.shard_map import shard_map
    from jax.sharding import Mesh, PartitionSpec, NamedSharding
    from concourse import bass2jax, mybir as _mybir

    x = np.asarray(inputs["x"], np.float32)
    c, arrs = _prep(inputs)
    if body_reps != 1:
        c["body_reps"] = body_reps
    key = tuple(sorted(c.items()))
    if key not in _CACHE:
        _CACHE[key] = _build(c)
    nc = _CACHE[key]
    bass2jax.install_neuronx_cc_hook()

    base = {
        "tqT": _to_bf16(arrs["tqT"]),
        "toT": _to_bf16(arrs["toT"]),
        "w1u": arrs["w1u"].astype(np.float32),
    }
    if c["need_g1"]:
        base["g1v"] = arrs["g1"]
    if c["need_b1"]:
        base["b1v"] = arrs["b1"]
    if c["need_bt"]:
        base["btv"] = arrs["bt"].astype(np.float32)
    in_maps = [dict(base, x=np.ascontiguousarray(_to_bf16(x[i]))) for i in range(B)]

    partition_name = nc.partition_id_tensor.name if nc.partition_id_tensor else None
    in_names, out_names, out_avals, zero_outs = [], [], [], []
    for alloc in nc.m.functions[0].allocations:
        if not isinstance(alloc, mybir.MemoryLocationSet):
            continue
        name = alloc.memorylocations[0].name
        if alloc.kind == "ExternalInput":
            if name != partition_name:
                in_names.append(name)
        elif alloc.kind == "ExternalOutput":
            out_names.append(name)
            shape = tuple(alloc.tensor_shape)
            dtype = mybir.dt.np(alloc.dtype)
            out_avals.append(jax.core.ShapedArray(shape, dtype))
            zero_outs.append(np.zeros(shape, dtype))
    n_params = len(in_names)

    bind_names = list(in_names) + list(out_names)
    if partition_name is not None:
        bind_names.append(partition_name)

    def _body(*args):
        operands = list(args)
        pid = [bass2jax.partition_id_tensor()] if partition_name else []
        outs = bass2jax._bass_exec_p.bind(
            *(operands + pid),
            out_avals=tuple(out_avals),
            in_names=tuple(bind_names),
            out_names=tuple(out_names),
            lowering_input_output_aliases=(),
            sim_require_finite=True,
            sim_require_nnan=True,
            nc=nc,
        )
        return tuple(outs)

    devices = jax.devices()[:B]
    mesh = Mesh(np.asarray(devices), ("core",))
    spec = PartitionSpec("core")
    n_out = len(out_names)
    per_core = [[np.asarray(m[nm]) for nm in in_names] for m in in_maps]
    concat_in = [
        np.concatenate([per_core[cc][i] for cc in range(B)], axis=0)
        for i in range(n_params)
    ]
    concat_zeros = [
        np.zeros((B * z.shape[0], *z.shape[1:]), z.dtype) for z in zero_outs
    ]
    dev_args = [
        jax.device_put(a, NamedSharding(mesh, spec)) for a in concat_in + concat_zeros
    ]

    f = jax.jit(
        shard_map(
            _body, mesh=mesh,
            in_specs=(spec,) * (n_params + n_out),
            out_specs=(spec,) * n_out,
            check_rep=False,
        )
    )
    jax.block_until_ready(f(*dev_args))  # compile + warm

    times = {}
    for k in (1, iters):
        best = float("inf")
        for _ in range(reps):
            t0 = _time.perf_counter()
            r = None
            for _ in range(k):
                r = f(*dev_args)  # async dispatch; device executes in-order
            jax.block_until_ready(r)
            best = min(best, _time.perf_counter() - t0)
        times[k] = best
    exec_ns = (times[iters] - times[1]) / (iters - 1) * 1e9
    return exec_ns, times

